# revision 23
# baseline (speedup 1.0000x reference)
"""CausalPrefixAttention TRN2 Bass kernel (v3).

Full-input contract: kernel(**inputs) takes the complete tensors and returns
the complete [2, 1024, 1024] output. Internally shards (batch, head-group)
across 8 NeuronCores: core c handles batch c//4 and heads 4*(c%4) .. +4.

v3 changes vs v2 (165.6us baseline):
- PE warmup matmuls at t=0 so HAM un-throttles before real work.
- DMA issue spread across engine queues (each dma_start costs ~610ns of
  queue issue time; v2 serialized ~22 of them on sync).
- Context mask folded multiplicatively into the V tiles (V rows and the
  denominator ones-row), removing the per-jc exp bias -> exp calls can be
  batched: 3 big ACTIVATEs per head instead of 16 (saves the 352-cycle
  per-instruction ACT overhead; ACT is the attention-phase bottleneck).
- sim tiles for the two heads of a pair issued adjacently: K=64 matmuls
  auto-derive tile_position (0,0)/(64,0) -> concurrent row-tiled execution.
- sim PSUM evacuated by DVE into an fp16 staging ring (full-precision f32
  logits -> fp16), exp reads SBUF (64K free-dim limit, not 4K).
- Denominator reciprocal via reciprocal_approx_fast (1 custom DVE op,
  ~5x faster than the 8-pass reciprocal).
- pair-1 K/Q projections interleaved into pair-0's ACT-bound attention
  window; out-projection of token tiles 0-3 starts as soon as both pairs'
  first-half denominators are ready.
"""

import sys

for _p in ("/opt/trn_rl_repo", "/root/.axon_site/_ro/trn_rl_repo"):
    if _p not in sys.path:
        sys.path.append(_p)

import numpy as np
import ml_dtypes

import concourse.bass as bass
import concourse.mybir as mybir
import concourse.tile as tile
from concourse import bacc, bass_utils


def _install_ntff_hook():
    """Provide antenv.axon_hooks (NTFF profiling shim) if the image lacks it."""
    try:
        from antenv import axon_hooks  # noqa: F401
        return
    except ImportError:
        pass
    import contextlib
    import ctypes
    import os
    import types

    so_path = "/opt/axon/libaxon_pjrt.so"
    hook = None
    if os.path.exists(so_path):
        lib = ctypes.CDLL(so_path)
        if hasattr(lib, "axon_start_nrt_profile"):
            lib.axon_start_nrt_profile.argtypes = [
                ctypes.POINTER(ctypes.c_int64), ctypes.c_size_t]
            lib.axon_start_nrt_profile.restype = ctypes.c_int64
            lib.axon_stop_nrt_profile.argtypes = [ctypes.c_char_p]
            lib.axon_stop_nrt_profile.restype = ctypes.c_int64

            @contextlib.contextmanager
            def hook(output_dir, device_ids):
                import jax
                jax.devices()
                if device_ids:
                    ids = (ctypes.c_int64 * len(device_ids))(*device_ids)
                    rc = lib.axon_start_nrt_profile(ids, len(device_ids))
                else:
                    rc = lib.axon_start_nrt_profile(None, 0)
                if rc != 0:
                    raise RuntimeError(f"axon_start_nrt_profile rc={rc}")
                try:
                    yield
                finally:
                    n = lib.axon_stop_nrt_profile(str(output_dir).encode())
                    print(f"ntff profile: {n} file(s) -> {output_dir}")

    mod = types.ModuleType("antenv.axon_hooks")
    mod.get_axon_ntff_profile_hook = lambda: hook
    mod.set_axon_ntff_profile_hook = lambda h: None
    sys.modules["antenv.axon_hooks"] = mod


_install_ntff_hook()

F32 = mybir.dt.float32
F16 = mybir.dt.float16
BF16 = mybir.dt.bfloat16
U8 = mybir.dt.uint8
AF = mybir.ActivationFunctionType
ALU = mybir.AluOpType

DIM = 1024
HEADS = 16
DH = 64
B = 2
N = 1024          # query tokens
CTX = 1024        # context tokens
J = CTX + N       # kv length
HPC = 4           # heads per core
INNER_C = HPC * DH  # 256 per-core inner width
SCALE = DH ** -0.5
LN_EPS = 1e-5

N_CORES = 8
NT = N // 128      # 8 query-token tiles
JT = J // 128      # 16 kv tiles
DT = DIM // 128    # 8 d-chunks

# per-jc valid widths (queries lo(jc)..1024) and packed column offsets
LO = [0 if jc <= 8 else 128 * (jc - 8) for jc in range(JT)]
W = [N - LO[jc] for jc in range(JT)]
OFF = [0] * JT
for _jc in range(1, JT):
    OFF[_jc] = OFF[_jc - 1] + W[_jc - 1]
TOTW = OFF[-1] + W[-1]          # 12800 packed columns per head
# exp chunks (jc ranges): 4096/4096/3328/1280 packed cols
CHUNKS = [(0, 4), (4, 8), (8, 12), (12, 16)]
SLOTW = 4096


def _build_program():
    nc = bacc.Bacc(
        "TRN2",
        target_bir_lowering=False,
        debug=False,
        enable_asserts=False,
        num_devices=N_CORES,
    )
    # normalized activations, d-major: chunk dc is [128, J] = x̂T rows 128dc..
    xt = nc.dram_tensor("xt", [128, DT * J], BF16, kind="ExternalInput").ap()
    # weights packed partition-major on host: [128, DT*INNER_C]
    wq = nc.dram_tensor("wq", [128, DT * INNER_C], BF16, kind="ExternalInput").ap()
    wk = nc.dram_tensor("wk", [128, DT * INNER_C], BF16, kind="ExternalInput").ap()
    wv = nc.dram_tensor("wv", [128, DT * INNER_C], BF16, kind="ExternalInput").ap()
    wo = nc.dram_tensor("wo", [128, 2 * DIM], BF16, kind="ExternalInput").ap()
    # rope tables, d-major [128 = 2x(2x32) dh, J]; ssin has sign folded
    cosd = nc.dram_tensor("cosd", [128, J], BF16, kind="ExternalInput").ap()
    ssind = nc.dram_tensor("ssind", [128, J], BF16, kind="ExternalInput").ap()
    cmask = nc.dram_tensor("cmask", [128, CTX // 128], U8, kind="ExternalInput").ap()
    y = nc.dram_tensor("y", [N, DIM], F32, kind="ExternalOutput").ap()

    with tile.TileContext(nc) as tc:
        _kernel_body(tc, xt, wq, wk, wv, wo, cosd, ssind, cmask, y)
    nc.finalize()
    return nc


def _kernel_body(tc, xt, wq, wk, wv, wo, cosd, ssind, cmask, y):
    nc = tc.nc
    ctx_lp = nc.allow_low_precision(reason="bf16 matmul operands; fp32 PSUM accumulation")
    ctx_lp.__enter__()
    mm = nc.tensor.matmul

    with (
        tc.tile_pool(name="consts", bufs=1) as cpool,
        tc.tile_pool(name="qkv", bufs=1) as qkv_pool,
        tc.tile_pool(name="ptbuf", bufs=1) as pt_pool,
        tc.tile_pool(name="woin", bufs=1) as woin_pool,
        tc.tile_pool(name="outsb", bufs=2) as out_pool,
        tc.tile_pool(name="ropetmp", bufs=2) as rp_pool,
        tc.tile_pool(name="dens", bufs=1) as dens_pool,
    ):
        # ---- tiny consts (no DMA deps) -----------------------------------
        onespc = cpool.tile([128, HPC], F32, tag="onespc", name="onespc")
        nc.vector.memset(onespc[:], 1.0)
        warmc = cpool.tile([128, 512], BF16, tag="warmc", name="warmc")
        nc.gpsimd.memset(warmc[:], 0.25)
        # denominator-broadcast selector: row 0 -> partitions 0:64, row 32 -> 64:128
        sel2f = cpool.tile([64, 128], F32, tag="sel2f", name="sel2f")
        nc.vector.memset(sel2f[:], 0.0)
        nc.vector.memset(sel2f[0:1, 0:64], 1.0)
        nc.vector.memset(sel2f[32:33, 64:128], 1.0)
        sel2 = cpool.tile([64, 128], BF16, tag="sel2", name="sel2")
        nc.vector.tensor_copy(sel2[:], sel2f[:])
        # dens rows (f32): rows 0/32 valid per use; init 1.0 so rcp of the
        # unused rows stays finite
        dens = dens_pool.tile([64, N], F32, tag="dens", name="dens")
        nc.vector.memset(dens[:], 1.0)
        rcpf = dens_pool.tile([64, N], F32, tag="rcpf", name="rcpf")
        nc.vector.memset(rcpf[:], 1.0)
        rcpb = dens_pool.tile([64, N], BF16, tag="rcpb", name="rcpb")

        # ---- PE warmup: ~10 junk matmuls flip HAM to 8/8 by ~3.5us -------
        with tc.tile_pool(name="warm_psum", bufs=1, space="PSUM") as wpsum:
            wps = wpsum.tile([128, 512], F32, tag="wps", name="wps")
            for _ in range(40):
                mm(wps[:], warmc[:, 0:128], warmc[:], start=True, stop=True)

        # ---- DMA issue: all on sync, in priority order -------------------
        # (concurrent dma_starts fair-share HBM bandwidth across queues, so
        #  the critical-path transfers must be issued first)
        # projection-phase inputs live in a manually-scoped pool that is
        # released after pair-0 attention so pair-1's P buffer can reuse it
        proj_cm = tc.tile_pool(name="projbuf", bufs=1)
        proj_pool = proj_cm.__enter__()
        mu = cpool.tile([128, CTX // 128], U8, tag="mu8", name="mu8")
        nc.sync.dma_start(mu[:], cmask[:])
        wvt = proj_pool.tile([128, DT * INNER_C], BF16, tag="wvt", name="wvt")
        nc.sync.dma_start(wvt[:], wv[:])
        # xt halves: one 3D-AP DMA each ([128, dc, 1024])
        xth = [proj_pool.tile([128, DT * 1024], BF16, tag=f"xth{hf}",
                              name=f"xth{hf}") for hf in range(2)]
        nc.sync.dma_start(xth[0][:], xt[:, 0:DT * 1024])
        wkt = proj_pool.tile([128, DT * INNER_C], BF16, tag="wkt", name="wkt")
        nc.sync.dma_start(wkt[:], wk[:])
        cosT = proj_pool.tile([128, J], BF16, tag="cosT", name="cosT")
        nc.sync.dma_start(cosT[:], cosd[:])
        ssinT = proj_pool.tile([128, J], BF16, tag="ssinT", name="ssinT")
        nc.sync.dma_start(ssinT[:], ssind[:])
        nc.sync.dma_start(xth[1][:], xt[:, DT * 1024:DT * 2048])
        wqt = proj_pool.tile([128, DT * INNER_C], BF16, tag="wqt", name="wqt")
        nc.sync.dma_start(wqt[:], wq[:])
        wot = cpool.tile([128, 2 * DIM], BF16, tag="wot", name="wot")
        nc.sync.dma_start(wot[:], wo[:])
        wo_t = [wot[:, DIM * i:DIM * (i + 1)] for i in range(2)]

        maskf = cpool.tile([128, CTX // 128], F32, tag="maskf", name="maskf")
        nc.vector.tensor_copy(maskf[:], mu[:])

        def xt_view(dc, col0, width):
            hf, off = divmod(col0, 1024)
            assert off + width <= 1024
            return xth[hf][:, 1024 * dc + off:1024 * dc + off + width]

        # ---- long-lived activation tiles --------------------------------
        qT = [qkv_pool.tile([128, N], BF16, tag=f"qT{i}", name=f"qT{i}") for i in range(2)]
        kT = [qkv_pool.tile([128, J], BF16, tag=f"kT{i}", name=f"kT{i}") for i in range(2)]
        vaug = [qkv_pool.tile([128, HPC * (DH + 1)], BF16, tag=f"va{j}", name=f"va{j}")
                for j in range(JT)]
        woin = [woin_pool.tile([128, N], BF16, tag=f"woin{i}", name=f"woin{i}")
                for i in range(2)]
        # packed P (exp'd, bf16): head hh of the current pair at cols
        # [TOTW*hh, TOTW*(hh+1)); single tile so exp can write both heads
        # through one strided AP
        ptall_a = pt_pool.tile([128, 2 * TOTW], BF16, tag="ptall", name="ptall")
        pt_a = (ptall_a, ptall_a[:].rearrange("p (h c) -> p h c", h=2))

        # ---- phase P1: V projection (token-major) -----------------------
        # va[kv, h, 0:64] = V * mask(kv);  va[kv, h, 64] = mask(kv)
        with (
            tc.tile_pool(name="v_psum", bufs=3, space="PSUM") as v_psum,
        ):
            for m in range(JT):
                ps = v_psum.tile([128, INNER_C], F32, tag="vp", name="vp")
                for dc in range(DT):
                    mm(ps[:], xt_view(dc, 128 * m, 128),
                       wvt[:, INNER_C * dc:INNER_C * (dc + 1)],
                       start=(dc == 0), stop=(dc == DT - 1))
                va = vaug[m][:].rearrange("p (h f) -> p h f", h=HPC)
                psv = ps[:].rearrange("p (h f) -> p h f", h=HPC)
                if m < CTX // 128:
                    nc.vector.tensor_scalar(
                        va[:, :, 0:DH], psv, scalar1=maskf[:, m:m + 1],
                        scalar2=None, op0=ALU.mult)
                    nc.vector.tensor_scalar(
                        va[:, :, DH:DH + 1],
                        onespc[:].rearrange("p (h o) -> p h o", o=1),
                        scalar1=maskf[:, m:m + 1], scalar2=None, op0=ALU.mult)
                else:
                    nc.vector.tensor_copy(va[:, :, 0:DH], psv)
                    nc.vector.tensor_copy(
                        va[:, :, DH:DH + 1],
                        onespc[:].rearrange("p (h o) -> p h o", o=1))

        # ---- K/Q projection + rope --------------------------------------
        def proj_rope(ps_alloc, w, ih, src0, pos0, dst, dst0):
            """d-major projection + rope into dst[:, dst0:dst0+N]."""
            psc = rp_pool.tile([128, N], BF16, tag="psc", name="psc")
            for h2 in range(2):
                ps = ps_alloc()
                for dc in range(DT):
                    mm(ps[:],
                       w[:, INNER_C * dc + 128 * ih:INNER_C * dc + 128 * (ih + 1)],
                       xt_view(dc, src0 + 512 * h2, 512),
                       start=(dc == 0), stop=(dc == DT - 1))
                # psum evac on DVE (keep ACT free for exp)
                nc.vector.tensor_copy(psc[:, 512 * h2:512 * (h2 + 1)], ps[:])
            c1 = rp_pool.tile([128, N], BF16, tag="c1", name="c1")
            nc.vector.tensor_mul(c1[:], psc[:], cosT[:, pos0:pos0 + N])
            ts = rp_pool.tile([128, N], BF16, tag="ts", name="ts")
            shuf_engs = (nc.sync, nc.scalar, nc.gpsimd, nc.sync)
            for blk in range(4):
                sb = blk ^ 1
                shuf_engs[blk].dma_start(ts[32 * blk:32 * (blk + 1), :],
                                         psc[32 * sb:32 * (sb + 1), :])
            c2 = rp_pool.tile([128, N], BF16, tag="c2", name="c2")
            nc.vector.tensor_mul(c2[:], ts[:], ssinT[:, pos0:pos0 + N])
            nc.vector.tensor_add(dst[:, dst0:dst0 + N], c1[:], c2[:])

        def kq_pair(ps_alloc, ih):
            proj_rope(ps_alloc, wqt, ih, CTX, CTX, qT[ih], 0)
            proj_rope(ps_alloc, wkt, ih, 0, 0, kT[ih], 0)
            proj_rope(ps_alloc, wkt, ih, N, N, kT[ih], N)

        # ---- attention helpers ------------------------------------------
        def attention_pair(ih, sim_psum, pv_psum, pt, interleave,
                           post_alpha=None, resim=False):
            """Attention for head pair ih. `interleave`: callables issued
            at jc 3/7/11; post_alpha: issued after the first-half
            normalization (jc 11)."""
            ktp, qtp = kT[ih], qT[ih]
            ptall, ptv = pt
            pvh = {}
            for hh in range(2):
                for nh in range(2):
                    pvh[(hh, nh)] = pv_psum.tile(
                        [65, 512], F32, tag=f"pv{hh}{nh}", name=f"pv{hh}{nh}")

            def pv_mm(hh, jc, nh):
                lo, off = LO[jc], OFF[jc]
                a = max(lo, 512 * nh)
                b = 512 * (nh + 1)
                if a >= b:
                    return
                h = 2 * ih + hh
                p0 = TOTW * hh + off
                mm(pvh[(hh, nh)][0:65, a - 512 * nh:b - 512 * nh],
                   vaug[jc][:, 65 * h:65 * h + 65],
                   ptall[:, p0 + (a - lo):p0 + (b - lo)],
                   start=(jc == 0),
                   stop=(jc == (11 if nh == 0 else 15)))

            def normalize(nh):
                cslice = slice(512 * nh, 512 * nh + 512)
                pvsb = rp_pool.tile([128, 512], F32, tag="pvsb", name="pvsb")
                for hh in range(2):
                    nc.vector.tensor_copy(pvsb[64 * hh:64 * hh + 64, :],
                                          pvh[(hh, nh)][0:64, :])
                    nc.vector.tensor_copy(dens[32 * hh:32 * hh + 1, cslice],
                                          pvh[(hh, nh)][64:65, :])
                nc.vector.reciprocal_approx_fast(rcpf[0:33, cslice],
                                                 dens[0:33, cslice])
                nc.vector.tensor_copy(rcpb[0:33, cslice], rcpf[0:33, cslice])
                bct = sim_psum.tile([128, 1024], F32, tag="simg", name="simg")
                bc = bct[:, 0:512]
                mm(bc, sel2[0:33, :], rcpb[0:33, cslice],
                   start=True, stop=True)
                nc.vector.tensor_mul(woin[ih][:, cslice], pvsb[:], bc)

            n_inter = 0
            for jc in range(JT):
                lo, w_ = LO[jc], W[jc]
                for seg0 in range(0, w_, 512):
                    seg1 = min(seg0 + 512, w_)
                    sw = seg1 - seg0
                    grp = sim_psum.tile([128, 1024], F32, tag="simg",
                                        name="simg")
                    gv = grp[:].rearrange("p (h c) -> p h c", h=2)
                    reps = 2 if resim else 1
                    for _ in range(reps):
                        for hh in range(2):
                            mm(grp[:, 512 * hh:512 * hh + sw],
                               ktp[64 * hh:64 * hh + 64,
                                   128 * jc:128 * (jc + 1)],
                               qtp[64 * hh:64 * hh + 64, lo + seg0:lo + seg1],
                               start=True, stop=True)
                    po = OFF[jc] + seg0
                    nc.scalar.activation(ptv[:, :, po:po + sw],
                                         gv[:, :, 0:sw], AF.Exp)
                    if jc >= 8 and seg0 == 0:
                        for hh in range(2):
                            nc.gpsimd.affine_select(
                                ptall[:, TOTW * hh + po:TOTW * hh + po + 128],
                                ptall[:, TOTW * hh + po:TOTW * hh + po + 128],
                                pattern=[[1, 128]], base=0,
                                channel_multiplier=-1,
                                compare_op=ALU.is_ge, fill=0.0)
                for hh in range(2):
                    if jc <= 11:
                        pv_mm(hh, jc, 0)
                    pv_mm(hh, jc, 1)

            normalize(0)
            normalize(1)

        # ---- out projection (per query-token tile m) --------------------
        def out_proj(sim_psum, m, evac):
            for nh in range(2):
                pst = sim_psum.tile([128, 1024], F32, tag="simg", name="simg")
                ps = pst[:, 0:512]
                for kc in range(2):
                    mm(ps,
                       woin[kc][:, 128 * m:128 * (m + 1)],
                       wo_t[kc][:, 512 * nh:512 * (nh + 1)],
                       start=(kc == 0), stop=(kc == 1))
                ot = out_pool.tile([128, 512], F32, tag="osb", name="osb")
                if evac == "scalar":
                    nc.scalar.copy(ot[:], ps)
                else:
                    nc.vector.tensor_copy(ot[:], ps)
                nc.gpsimd.dma_start(
                    y[128 * m:128 * (m + 1), 512 * nh:512 * (nh + 1)], ot[:])

        # ---- main schedule ----------------------------------------------
        with tc.tile_pool(name="qk_psum", bufs=2, space="PSUM") as qk_psum:
            kq_pair(lambda: qk_psum.tile([128, 512], F32, tag="qkp",
                                         name="qkp"), 0)
        with (
            tc.tile_pool(name="sim_psum", bufs=2, space="PSUM") as sim_psum,
            tc.tile_pool(name="pv_psum", bufs=1, space="PSUM") as pv_psum,
        ):
            sim_alloc = lambda: sim_psum.tile([128, 1024], F32, tag="simg",
                                              name="simg")[:, 0:512]
            attention_pair(0, sim_psum, pv_psum, pt_a, [], resim=True)
            kq_pair(sim_alloc, 1)
            proj_cm.__exit__(None, None, None)
            with tc.tile_pool(name="ptbuf2", bufs=1) as pt2_pool:
                ptall_b = pt2_pool.tile([128, 2 * TOTW], BF16, tag="ptallb",
                                        name="ptallb")
                pt_b = (ptall_b, ptall_b[:].rearrange("p (h c) -> p h c", h=2))
                attention_pair(1, sim_psum, pv_psum, pt_b, [], resim=True)
                for m in range(4):
                    out_proj(sim_psum, m, "vector")
                for m in range(4, NT):
                    out_proj(sim_psum, m, "scalar")
    ctx_lp.__exit__(None, None, None)


_NC = None
_LAST_RESULTS = None


def _get_program():
    global _NC
    if _NC is None:
        _NC = _build_program()
    return _NC


def _pack_rows(a):
    # [DT*128, W] -> [128, DT*W] partition-major
    k, w = a.shape[0] // 128, a.shape[1]
    return np.ascontiguousarray(
        a.reshape(k, 128, w).transpose(1, 0, 2).reshape(128, k * w))


def _bf16(a):
    return np.ascontiguousarray(a.astype(ml_dtypes.bfloat16))


def _ln(a, w, b):
    mu = a.mean(-1, keepdims=True)
    var = a.var(-1, keepdims=True)
    return (a - mu) / np.sqrt(var + LN_EPS) * w + b


def kernel(x, context, context_mask, rotary_pos_emb, norm_w, norm_b,
           cnorm_w, cnorm_b, Wq, Wkv, Wo, bo, _trace=False):
    global _LAST_RESULTS
    x = np.asarray(x, dtype=np.float32)
    context = np.asarray(context, dtype=np.float32)
    rot = np.asarray(rotary_pos_emb, dtype=np.float32)

    xn = _ln(x, np.asarray(norm_w, np.float32), np.asarray(norm_b, np.float32))
    cn = _ln(context, np.asarray(cnorm_w, np.float32),
             np.asarray(cnorm_b, np.float32))
    # [b] -> [128, DT*J] d-major packed bf16
    xt_pk = []
    for b in range(B):
        allx = np.concatenate([cn[b], xn[b]], axis=0)       # [J, DIM]
        pk = _pack_rows(np.ascontiguousarray(allx.T))       # [128, DT*J]
        # reorder to half-major: [h0: dc0..7 | h1: dc0..7], contiguous DMAs
        pk = pk.reshape(128, DT, 2, 1024).transpose(0, 2, 1, 3).reshape(
            128, DT * J)
        xt_pk.append(_bf16(pk))

    # rope tables d-major with sign folded into ssin
    cosT = np.tile(np.cos(rot).T, (2, 1))                   # [128, J]
    ssinT = np.sin(rot).T.copy()
    ssinT[:32] *= -1.0
    ssinT = np.tile(ssinT, (2, 1))
    cosT = _bf16(cosT)
    ssinT = _bf16(ssinT)

    Wq = np.asarray(Wq, dtype=np.float32) * SCALE
    Wkv = np.asarray(Wkv, dtype=np.float32)
    Wo = np.asarray(Wo, dtype=np.float32)
    mask_u8 = np.asarray(context_mask).reshape(B, CTX // 128, 128).view(np.uint8)
    mask_u8 = [np.ascontiguousarray(mask_u8[b].T) for b in range(B)]

    in_maps = []
    for c in range(N_CORES):
        b, hg = divmod(c, HEADS // HPC)
        lo = DH * HPC * hg
        in_maps.append({
            "xt": xt_pk[b],
            "wq": _bf16(_pack_rows(Wq[:, lo:lo + INNER_C])),
            "wk": _bf16(_pack_rows(Wkv[:, lo:lo + INNER_C])),
            "wv": _bf16(_pack_rows(Wkv[:, HEADS * DH + lo:HEADS * DH + lo + INNER_C])),
            "wo": _bf16(_pack_rows(Wo[lo:lo + INNER_C, :])),
            "cosd": cosT, "ssind": ssinT,
            "cmask": mask_u8[b],
        })

    nc = _get_program()
    res = bass_utils.run_bass_kernel_spmd(
        nc, in_maps, core_ids=list(range(N_CORES)), trace=_trace,
    )
    _LAST_RESULTS = res
    out = np.zeros((B, N, DIM), dtype=np.float32)
    for c in range(N_CORES):
        out[c // (HEADS // HPC)] += res.results[c]["y"]
    out += np.asarray(bo, dtype=np.float32)
    return out


# revision 24
# speedup vs baseline: 1.2003x; 1.2003x over previous
"""CausalPrefixAttention TRN2 Bass kernel (v3).

Full-input contract: kernel(**inputs) takes the complete tensors and returns
the complete [2, 1024, 1024] output. Internally shards (batch, head-group)
across 8 NeuronCores: core c handles batch c//4 and heads 4*(c%4) .. +4.

v3 changes vs v2 (165.6us baseline):
- PE warmup matmuls at t=0 so HAM un-throttles before real work.
- DMA issue spread across engine queues (each dma_start costs ~610ns of
  queue issue time; v2 serialized ~22 of them on sync).
- Context mask folded multiplicatively into the V tiles (V rows and the
  denominator ones-row), removing the per-jc exp bias -> exp calls can be
  batched: 3 big ACTIVATEs per head instead of 16 (saves the 352-cycle
  per-instruction ACT overhead; ACT is the attention-phase bottleneck).
- sim tiles for the two heads of a pair issued adjacently: K=64 matmuls
  auto-derive tile_position (0,0)/(64,0) -> concurrent row-tiled execution.
- sim PSUM evacuated by DVE into an fp16 staging ring (full-precision f32
  logits -> fp16), exp reads SBUF (64K free-dim limit, not 4K).
- Denominator reciprocal via reciprocal_approx_fast (1 custom DVE op,
  ~5x faster than the 8-pass reciprocal).
- pair-1 K/Q projections interleaved into pair-0's ACT-bound attention
  window; out-projection of token tiles 0-3 starts as soon as both pairs'
  first-half denominators are ready.
"""

import sys

for _p in ("/opt/trn_rl_repo", "/root/.axon_site/_ro/trn_rl_repo"):
    if _p not in sys.path:
        sys.path.append(_p)

import numpy as np
import ml_dtypes

import concourse.bass as bass
import concourse.mybir as mybir
import concourse.tile as tile
from concourse import bacc, bass_utils


def _install_ntff_hook():
    """Provide antenv.axon_hooks (NTFF profiling shim) if the image lacks it."""
    try:
        from antenv import axon_hooks  # noqa: F401
        return
    except ImportError:
        pass
    import contextlib
    import ctypes
    import os
    import types

    so_path = "/opt/axon/libaxon_pjrt.so"
    hook = None
    if os.path.exists(so_path):
        lib = ctypes.CDLL(so_path)
        if hasattr(lib, "axon_start_nrt_profile"):
            lib.axon_start_nrt_profile.argtypes = [
                ctypes.POINTER(ctypes.c_int64), ctypes.c_size_t]
            lib.axon_start_nrt_profile.restype = ctypes.c_int64
            lib.axon_stop_nrt_profile.argtypes = [ctypes.c_char_p]
            lib.axon_stop_nrt_profile.restype = ctypes.c_int64

            @contextlib.contextmanager
            def hook(output_dir, device_ids):
                import jax
                jax.devices()
                if device_ids:
                    ids = (ctypes.c_int64 * len(device_ids))(*device_ids)
                    rc = lib.axon_start_nrt_profile(ids, len(device_ids))
                else:
                    rc = lib.axon_start_nrt_profile(None, 0)
                if rc != 0:
                    raise RuntimeError(f"axon_start_nrt_profile rc={rc}")
                try:
                    yield
                finally:
                    n = lib.axon_stop_nrt_profile(str(output_dir).encode())
                    print(f"ntff profile: {n} file(s) -> {output_dir}")

    mod = types.ModuleType("antenv.axon_hooks")
    mod.get_axon_ntff_profile_hook = lambda: hook
    mod.set_axon_ntff_profile_hook = lambda h: None
    sys.modules["antenv.axon_hooks"] = mod


_install_ntff_hook()

F32 = mybir.dt.float32
F16 = mybir.dt.float16
BF16 = mybir.dt.bfloat16
U8 = mybir.dt.uint8
AF = mybir.ActivationFunctionType
ALU = mybir.AluOpType

DIM = 1024
HEADS = 16
DH = 64
B = 2
N = 1024          # query tokens
CTX = 1024        # context tokens
J = CTX + N       # kv length
HPC = 4           # heads per core
INNER_C = HPC * DH  # 256 per-core inner width
SCALE = DH ** -0.5
LN_EPS = 1e-5

N_CORES = 8
NT = N // 128      # 8 query-token tiles
JT = J // 128      # 16 kv tiles
DT = DIM // 128    # 8 d-chunks

# per-jc valid widths (queries lo(jc)..1024) and packed column offsets
LO = [0 if jc <= 8 else 128 * (jc - 8) for jc in range(JT)]
W = [N - LO[jc] for jc in range(JT)]
OFF = [0] * JT
for _jc in range(1, JT):
    OFF[_jc] = OFF[_jc - 1] + W[_jc - 1]
TOTW = OFF[-1] + W[-1]          # 12800 packed columns per head
# exp chunks (jc ranges): 4096/4096/3328/1280 packed cols
CHUNKS = [(0, 4), (4, 8), (8, 12), (12, 16)]
SLOTW = 4096


def _build_program():
    nc = bacc.Bacc(
        "TRN2",
        target_bir_lowering=False,
        debug=False,
        enable_asserts=False,
        num_devices=N_CORES,
    )
    # normalized activations, d-major: chunk dc is [128, J] = x̂T rows 128dc..
    xt = nc.dram_tensor("xt", [128, DT * J], BF16, kind="ExternalInput").ap()
    # weights packed partition-major on host: [128, DT*INNER_C]
    wq = nc.dram_tensor("wq", [128, DT * INNER_C], BF16, kind="ExternalInput").ap()
    wk = nc.dram_tensor("wk", [128, DT * INNER_C], BF16, kind="ExternalInput").ap()
    wv = nc.dram_tensor("wv", [128, DT * INNER_C], BF16, kind="ExternalInput").ap()
    wo = nc.dram_tensor("wo", [128, 2 * DIM], BF16, kind="ExternalInput").ap()
    # rope tables, d-major [128 = 2x(2x32) dh, J]; ssin has sign folded
    cosd = nc.dram_tensor("cosd", [128, J], BF16, kind="ExternalInput").ap()
    ssind = nc.dram_tensor("ssind", [128, J], BF16, kind="ExternalInput").ap()
    cmask = nc.dram_tensor("cmask", [128, CTX // 128], U8, kind="ExternalInput").ap()
    y = nc.dram_tensor("y", [N, DIM], F32, kind="ExternalOutput").ap()

    with tile.TileContext(nc) as tc:
        _kernel_body(tc, xt, wq, wk, wv, wo, cosd, ssind, cmask, y)
    nc.finalize()
    return nc


def _kernel_body(tc, xt, wq, wk, wv, wo, cosd, ssind, cmask, y):
    nc = tc.nc
    ctx_lp = nc.allow_low_precision(reason="bf16 matmul operands; fp32 PSUM accumulation")
    ctx_lp.__enter__()
    mm = nc.tensor.matmul

    with (
        tc.tile_pool(name="consts", bufs=1) as cpool,
        tc.tile_pool(name="qkv", bufs=1) as qkv_pool,
        tc.tile_pool(name="ptbuf", bufs=1) as pt_pool,
        tc.tile_pool(name="woin", bufs=1) as woin_pool,
        tc.tile_pool(name="outsb", bufs=2) as out_pool,
        tc.tile_pool(name="ropetmp", bufs=2) as rp_pool,
        tc.tile_pool(name="dens", bufs=1) as dens_pool,
    ):
        # ---- tiny consts (no DMA deps) -----------------------------------
        onespc = cpool.tile([128, HPC], F32, tag="onespc", name="onespc")
        nc.vector.memset(onespc[:], 1.0)
        warmc = cpool.tile([128, 512], BF16, tag="warmc", name="warmc")
        nc.gpsimd.memset(warmc[:], 0.25)
        # denominator-broadcast selector: row 0 -> partitions 0:64, row 32 -> 64:128
        sel2f = cpool.tile([64, 128], F32, tag="sel2f", name="sel2f")
        nc.vector.memset(sel2f[:], 0.0)
        nc.vector.memset(sel2f[0:1, 0:64], 1.0)
        nc.vector.memset(sel2f[32:33, 64:128], 1.0)
        sel2 = cpool.tile([64, 128], BF16, tag="sel2", name="sel2")
        nc.vector.tensor_copy(sel2[:], sel2f[:])
        # dens rows (f32): rows 0/32 valid per use; init 1.0 so rcp of the
        # unused rows stays finite
        dens = dens_pool.tile([64, N], F32, tag="dens", name="dens")
        nc.vector.memset(dens[:], 1.0)
        rcpf = dens_pool.tile([64, N], F32, tag="rcpf", name="rcpf")
        nc.vector.memset(rcpf[:], 1.0)
        rcpb = dens_pool.tile([64, N], BF16, tag="rcpb", name="rcpb")

        # ---- PE warmup: ~10 junk matmuls flip HAM to 8/8 by ~3.5us -------
        with tc.tile_pool(name="warm_psum", bufs=1, space="PSUM") as wpsum:
            wps = wpsum.tile([128, 512], F32, tag="wps", name="wps")
            for _ in range(40):
                mm(wps[:], warmc[:, 0:128], warmc[:], start=True, stop=True)

        # ---- DMA issue: all on sync, in priority order -------------------
        # (concurrent dma_starts fair-share HBM bandwidth across queues, so
        #  the critical-path transfers must be issued first)
        # projection-phase inputs live in a manually-scoped pool that is
        # released after pair-0 attention so pair-1's P buffer can reuse it
        proj_cm = tc.tile_pool(name="projbuf", bufs=1)
        proj_pool = proj_cm.__enter__()
        mu = cpool.tile([128, CTX // 128], U8, tag="mu8", name="mu8")
        nc.sync.dma_start(mu[:], cmask[:])
        wvt = proj_pool.tile([128, DT * INNER_C], BF16, tag="wvt", name="wvt")
        nc.sync.dma_start(wvt[:], wv[:])
        # xt halves: one 3D-AP DMA each ([128, dc, 1024])
        xth = [proj_pool.tile([128, DT * 1024], BF16, tag=f"xth{hf}",
                              name=f"xth{hf}") for hf in range(2)]
        nc.sync.dma_start(xth[0][:], xt[:, 0:DT * 1024])
        wkt = proj_pool.tile([128, DT * INNER_C], BF16, tag="wkt", name="wkt")
        nc.sync.dma_start(wkt[:], wk[:])
        cosT = proj_pool.tile([128, J], BF16, tag="cosT", name="cosT")
        nc.sync.dma_start(cosT[:], cosd[:])
        ssinT = proj_pool.tile([128, J], BF16, tag="ssinT", name="ssinT")
        nc.sync.dma_start(ssinT[:], ssind[:])
        nc.sync.dma_start(xth[1][:], xt[:, DT * 1024:DT * 2048])
        wqt = proj_pool.tile([128, DT * INNER_C], BF16, tag="wqt", name="wqt")
        nc.sync.dma_start(wqt[:], wq[:])
        wot = cpool.tile([128, 2 * DIM], BF16, tag="wot", name="wot")
        nc.sync.dma_start(wot[:], wo[:])
        wo_t = [wot[:, DIM * i:DIM * (i + 1)] for i in range(2)]

        maskf = cpool.tile([128, CTX // 128], F32, tag="maskf", name="maskf")
        nc.vector.tensor_copy(maskf[:], mu[:])

        def xt_view(dc, col0, width):
            hf, off = divmod(col0, 1024)
            assert off + width <= 1024
            return xth[hf][:, 1024 * dc + off:1024 * dc + off + width]

        # ---- long-lived activation tiles --------------------------------
        qT = [qkv_pool.tile([128, N], BF16, tag=f"qT{i}", name=f"qT{i}") for i in range(2)]
        kT = [qkv_pool.tile([128, J], BF16, tag=f"kT{i}", name=f"kT{i}") for i in range(2)]
        vaug = [qkv_pool.tile([128, HPC * (DH + 1)], BF16, tag=f"va{j}", name=f"va{j}")
                for j in range(JT)]
        woin = [woin_pool.tile([128, N], BF16, tag=f"woin{i}", name=f"woin{i}")
                for i in range(2)]
        # packed P (exp'd, bf16): head hh of the current pair at cols
        # [TOTW*hh, TOTW*(hh+1)); single tile so exp can write both heads
        # through one strided AP
        ptall_a = pt_pool.tile([128, 2 * TOTW], BF16, tag="ptall", name="ptall")
        pt_a = (ptall_a, ptall_a[:].rearrange("p (h c) -> p h c", h=2))

        # ---- phase P1: V projection (token-major) -----------------------
        # va[kv, h, 0:64] = V * mask(kv);  va[kv, h, 64] = mask(kv)
        with (
            tc.tile_pool(name="v_psum", bufs=3, space="PSUM") as v_psum,
        ):
            for m in range(JT):
                ps = v_psum.tile([128, INNER_C], F32, tag="vp", name="vp")
                for dc in range(DT):
                    mm(ps[:], xt_view(dc, 128 * m, 128),
                       wvt[:, INNER_C * dc:INNER_C * (dc + 1)],
                       start=(dc == 0), stop=(dc == DT - 1))
                va = vaug[m][:].rearrange("p (h f) -> p h f", h=HPC)
                psv = ps[:].rearrange("p (h f) -> p h f", h=HPC)
                if m < CTX // 128:
                    nc.vector.tensor_scalar(
                        va[:, :, 0:DH], psv, scalar1=maskf[:, m:m + 1],
                        scalar2=None, op0=ALU.mult)
                    nc.vector.tensor_scalar(
                        va[:, :, DH:DH + 1],
                        onespc[:].rearrange("p (h o) -> p h o", o=1),
                        scalar1=maskf[:, m:m + 1], scalar2=None, op0=ALU.mult)
                else:
                    nc.vector.tensor_copy(va[:, :, 0:DH], psv)
                    nc.vector.tensor_copy(
                        va[:, :, DH:DH + 1],
                        onespc[:].rearrange("p (h o) -> p h o", o=1))

        # ---- K/Q projection + rope --------------------------------------
        def proj_rope(ps_alloc, w, ih, src0, pos0, dst, dst0):
            """d-major projection + rope into dst[:, dst0:dst0+N]."""
            psc = rp_pool.tile([128, N], BF16, tag="psc", name="psc")
            for h2 in range(2):
                ps = ps_alloc()
                for dc in range(DT):
                    mm(ps[:],
                       w[:, INNER_C * dc + 128 * ih:INNER_C * dc + 128 * (ih + 1)],
                       xt_view(dc, src0 + 512 * h2, 512),
                       start=(dc == 0), stop=(dc == DT - 1))
                # psum evac on DVE (keep ACT free for exp)
                nc.vector.tensor_copy(psc[:, 512 * h2:512 * (h2 + 1)], ps[:])
            c1 = rp_pool.tile([128, N], BF16, tag="c1", name="c1")
            nc.vector.tensor_mul(c1[:], psc[:], cosT[:, pos0:pos0 + N])
            ts = rp_pool.tile([128, N], BF16, tag="ts", name="ts")
            shuf_engs = (nc.sync, nc.scalar, nc.gpsimd, nc.sync)
            for blk in range(4):
                sb = blk ^ 1
                shuf_engs[blk].dma_start(ts[32 * blk:32 * (blk + 1), :],
                                         psc[32 * sb:32 * (sb + 1), :])
            c2 = rp_pool.tile([128, N], BF16, tag="c2", name="c2")
            nc.vector.tensor_mul(c2[:], ts[:], ssinT[:, pos0:pos0 + N])
            nc.vector.tensor_add(dst[:, dst0:dst0 + N], c1[:], c2[:])

        def kq_pair(ps_alloc, ih):
            proj_rope(ps_alloc, wqt, ih, CTX, CTX, qT[ih], 0)
            proj_rope(ps_alloc, wkt, ih, 0, 0, kT[ih], 0)
            proj_rope(ps_alloc, wkt, ih, N, N, kT[ih], N)

        # ---- attention helpers ------------------------------------------
        def attention_pair(ih, sim_psum, pv_psum, pt, interleave,
                           post_alpha=None, resim=False):
            """Attention for head pair ih. `interleave`: callables issued
            at jc 3/7/11; post_alpha: issued after the first-half
            normalization (jc 11)."""
            ktp, qtp = kT[ih], qT[ih]
            ptall, ptv = pt
            pvh = {}
            for hh in range(2):
                for nh in range(2):
                    pvh[(hh, nh)] = pv_psum.tile(
                        [65, 512], F32, tag=f"pv{hh}{nh}", name=f"pv{hh}{nh}")

            def pv_mm(hh, jc, nh):
                lo, off = LO[jc], OFF[jc]
                a = max(lo, 512 * nh)
                b = 512 * (nh + 1)
                if a >= b:
                    return
                h = 2 * ih + hh
                p0 = TOTW * hh + off
                mm(pvh[(hh, nh)][0:65, a - 512 * nh:b - 512 * nh],
                   vaug[jc][:, 65 * h:65 * h + 65],
                   ptall[:, p0 + (a - lo):p0 + (b - lo)],
                   start=(jc == 0),
                   stop=(jc == (11 if nh == 0 else 15)))

            def normalize(nh):
                cslice = slice(512 * nh, 512 * nh + 512)
                pvsb = rp_pool.tile([128, 512], F32, tag="pvsb", name="pvsb")
                for hh in range(2):
                    nc.vector.tensor_copy(pvsb[64 * hh:64 * hh + 64, :],
                                          pvh[(hh, nh)][0:64, :])
                    nc.vector.tensor_copy(dens[32 * hh:32 * hh + 1, cslice],
                                          pvh[(hh, nh)][64:65, :])
                nc.vector.reciprocal_approx_fast(rcpf[0:33, cslice],
                                                 dens[0:33, cslice])
                nc.vector.tensor_copy(rcpb[0:33, cslice], rcpf[0:33, cslice])
                bct = sim_psum.tile([128, 1024], F32, tag="simg", name="simg")
                bc = bct[:, 0:512]
                mm(bc, sel2[0:33, :], rcpb[0:33, cslice],
                   start=True, stop=True)
                nc.vector.tensor_mul(woin[ih][:, cslice], pvsb[:], bc)

            n_inter = 0
            for jc in range(JT):
                lo, w_ = LO[jc], W[jc]
                for seg0 in range(0, w_, 512):
                    seg1 = min(seg0 + 512, w_)
                    sw = seg1 - seg0
                    grp = sim_psum.tile([128, 1024], F32, tag="simg",
                                        name="simg")
                    gv = grp[:].rearrange("p (h c) -> p h c", h=2)
                    reps = 2 if resim else 1
                    for _ in range(reps):
                        for hh in range(2):
                            mm(grp[:, 512 * hh:512 * hh + sw],
                               ktp[64 * hh:64 * hh + 64,
                                   128 * jc:128 * (jc + 1)],
                               qtp[64 * hh:64 * hh + 64, lo + seg0:lo + seg1],
                               start=True, stop=True)
                    po = OFF[jc] + seg0
                    nc.scalar.activation(ptv[:, :, po:po + sw],
                                         gv[:, :, 0:sw], AF.Exp)
                    if jc >= 8 and seg0 == 0:
                        for hh in range(2):
                            nc.gpsimd.affine_select(
                                ptall[:, TOTW * hh + po:TOTW * hh + po + 128],
                                ptall[:, TOTW * hh + po:TOTW * hh + po + 128],
                                pattern=[[1, 128]], base=0,
                                channel_multiplier=-1,
                                compare_op=ALU.is_ge, fill=0.0)
                for hh in range(2):
                    if jc <= 11:
                        pv_mm(hh, jc, 0)
                    pv_mm(hh, jc, 1)

            normalize(0)
            normalize(1)

        # ---- out projection (per query-token tile m) --------------------
        def out_proj(sim_psum, m, evac):
            for nh in range(2):
                pst = sim_psum.tile([128, 1024], F32, tag="simg", name="simg")
                ps = pst[:, 0:512]
                for kc in range(2):
                    mm(ps,
                       woin[kc][:, 128 * m:128 * (m + 1)],
                       wo_t[kc][:, 512 * nh:512 * (nh + 1)],
                       start=(kc == 0), stop=(kc == 1))
                ot = out_pool.tile([128, 512], F32, tag="osb", name="osb")
                if evac == "scalar":
                    nc.scalar.copy(ot[:], ps)
                else:
                    nc.vector.tensor_copy(ot[:], ps)
                nc.gpsimd.dma_start(
                    y[128 * m:128 * (m + 1), 512 * nh:512 * (nh + 1)], ot[:])

        # ---- main schedule ----------------------------------------------
        with tc.tile_pool(name="qk_psum", bufs=2, space="PSUM") as qk_psum:
            kq_pair(lambda: qk_psum.tile([128, 512], F32, tag="qkp",
                                         name="qkp"), 0)
        with (
            tc.tile_pool(name="sim_psum", bufs=2, space="PSUM") as sim_psum,
            tc.tile_pool(name="pv_psum", bufs=1, space="PSUM") as pv_psum,
        ):
            sim_alloc = lambda: sim_psum.tile([128, 1024], F32, tag="simg",
                                              name="simg")[:, 0:512]
            attention_pair(0, sim_psum, pv_psum, pt_a, [])
            kq_pair(sim_alloc, 1)
            proj_cm.__exit__(None, None, None)
            with tc.tile_pool(name="ptbuf2", bufs=1) as pt2_pool:
                ptall_b = pt2_pool.tile([128, 2 * TOTW], BF16, tag="ptallb",
                                        name="ptallb")
                pt_b = (ptall_b, ptall_b[:].rearrange("p (h c) -> p h c", h=2))
                attention_pair(1, sim_psum, pv_psum, pt_b, [])
                for m in range(4):
                    out_proj(sim_psum, m, "vector")
                for m in range(4, NT):
                    out_proj(sim_psum, m, "scalar")
    ctx_lp.__exit__(None, None, None)


_NC = None
_LAST_RESULTS = None


def _get_program():
    global _NC
    if _NC is None:
        _NC = _build_program()
    return _NC


def _pack_rows(a):
    # [DT*128, W] -> [128, DT*W] partition-major
    k, w = a.shape[0] // 128, a.shape[1]
    return np.ascontiguousarray(
        a.reshape(k, 128, w).transpose(1, 0, 2).reshape(128, k * w))


def _bf16(a):
    return np.ascontiguousarray(a.astype(ml_dtypes.bfloat16))


def _ln(a, w, b):
    mu = a.mean(-1, keepdims=True)
    var = a.var(-1, keepdims=True)
    return (a - mu) / np.sqrt(var + LN_EPS) * w + b


def kernel(x, context, context_mask, rotary_pos_emb, norm_w, norm_b,
           cnorm_w, cnorm_b, Wq, Wkv, Wo, bo, _trace=False):
    global _LAST_RESULTS
    x = np.asarray(x, dtype=np.float32)
    context = np.asarray(context, dtype=np.float32)
    rot = np.asarray(rotary_pos_emb, dtype=np.float32)

    xn = _ln(x, np.asarray(norm_w, np.float32), np.asarray(norm_b, np.float32))
    cn = _ln(context, np.asarray(cnorm_w, np.float32),
             np.asarray(cnorm_b, np.float32))
    # [b] -> [128, DT*J] d-major packed bf16
    xt_pk = []
    for b in range(B):
        allx = np.concatenate([cn[b], xn[b]], axis=0)       # [J, DIM]
        pk = _pack_rows(np.ascontiguousarray(allx.T))       # [128, DT*J]
        # reorder to half-major: [h0: dc0..7 | h1: dc0..7], contiguous DMAs
        pk = pk.reshape(128, DT, 2, 1024).transpose(0, 2, 1, 3).reshape(
            128, DT * J)
        xt_pk.append(_bf16(pk))

    # rope tables d-major with sign folded into ssin
    cosT = np.tile(np.cos(rot).T, (2, 1))                   # [128, J]
    ssinT = np.sin(rot).T.copy()
    ssinT[:32] *= -1.0
    ssinT = np.tile(ssinT, (2, 1))
    cosT = _bf16(cosT)
    ssinT = _bf16(ssinT)

    Wq = np.asarray(Wq, dtype=np.float32) * SCALE
    Wkv = np.asarray(Wkv, dtype=np.float32)
    Wo = np.asarray(Wo, dtype=np.float32)
    mask_u8 = np.asarray(context_mask).reshape(B, CTX // 128, 128).view(np.uint8)
    mask_u8 = [np.ascontiguousarray(mask_u8[b].T) for b in range(B)]

    in_maps = []
    for c in range(N_CORES):
        b, hg = divmod(c, HEADS // HPC)
        lo = DH * HPC * hg
        in_maps.append({
            "xt": xt_pk[b],
            "wq": _bf16(_pack_rows(Wq[:, lo:lo + INNER_C])),
            "wk": _bf16(_pack_rows(Wkv[:, lo:lo + INNER_C])),
            "wv": _bf16(_pack_rows(Wkv[:, HEADS * DH + lo:HEADS * DH + lo + INNER_C])),
            "wo": _bf16(_pack_rows(Wo[lo:lo + INNER_C, :])),
            "cosd": cosT, "ssind": ssinT,
            "cmask": mask_u8[b],
        })

    nc = _get_program()
    res = bass_utils.run_bass_kernel_spmd(
        nc, in_maps, core_ids=list(range(N_CORES)), trace=_trace,
    )
    _LAST_RESULTS = res
    out = np.zeros((B, N, DIM), dtype=np.float32)
    for c in range(N_CORES):
        out[c // (HEADS // HPC)] += res.results[c]["y"]
    out += np.asarray(bo, dtype=np.float32)
    return out


# revision 25
# speedup vs baseline: 1.3555x; 1.1292x over previous
"""CausalPrefixAttention TRN2 Bass kernel (v3).

Full-input contract: kernel(**inputs) takes the complete tensors and returns
the complete [2, 1024, 1024] output. Internally shards (batch, head-group)
across 8 NeuronCores: core c handles batch c//4 and heads 4*(c%4) .. +4.

v3 changes vs v2 (165.6us baseline):
- PE warmup matmuls at t=0 so HAM un-throttles before real work.
- DMA issue spread across engine queues (each dma_start costs ~610ns of
  queue issue time; v2 serialized ~22 of them on sync).
- Context mask folded multiplicatively into the V tiles (V rows and the
  denominator ones-row), removing the per-jc exp bias -> exp calls can be
  batched: 3 big ACTIVATEs per head instead of 16 (saves the 352-cycle
  per-instruction ACT overhead; ACT is the attention-phase bottleneck).
- sim tiles for the two heads of a pair issued adjacently: K=64 matmuls
  auto-derive tile_position (0,0)/(64,0) -> concurrent row-tiled execution.
- sim PSUM evacuated by DVE into an fp16 staging ring (full-precision f32
  logits -> fp16), exp reads SBUF (64K free-dim limit, not 4K).
- Denominator reciprocal via reciprocal_approx_fast (1 custom DVE op,
  ~5x faster than the 8-pass reciprocal).
- pair-1 K/Q projections interleaved into pair-0's ACT-bound attention
  window; out-projection of token tiles 0-3 starts as soon as both pairs'
  first-half denominators are ready.
"""

import sys

for _p in ("/opt/trn_rl_repo", "/root/.axon_site/_ro/trn_rl_repo"):
    if _p not in sys.path:
        sys.path.append(_p)

import numpy as np
import ml_dtypes

import concourse.bass as bass
import concourse.mybir as mybir
import concourse.tile as tile
from concourse import bacc, bass_utils


def _install_ntff_hook():
    """Provide antenv.axon_hooks (NTFF profiling shim) if the image lacks it."""
    try:
        from antenv import axon_hooks  # noqa: F401
        return
    except ImportError:
        pass
    import contextlib
    import ctypes
    import os
    import types

    so_path = "/opt/axon/libaxon_pjrt.so"
    hook = None
    if os.path.exists(so_path):
        lib = ctypes.CDLL(so_path)
        if hasattr(lib, "axon_start_nrt_profile"):
            lib.axon_start_nrt_profile.argtypes = [
                ctypes.POINTER(ctypes.c_int64), ctypes.c_size_t]
            lib.axon_start_nrt_profile.restype = ctypes.c_int64
            lib.axon_stop_nrt_profile.argtypes = [ctypes.c_char_p]
            lib.axon_stop_nrt_profile.restype = ctypes.c_int64

            @contextlib.contextmanager
            def hook(output_dir, device_ids):
                import jax
                jax.devices()
                if device_ids:
                    ids = (ctypes.c_int64 * len(device_ids))(*device_ids)
                    rc = lib.axon_start_nrt_profile(ids, len(device_ids))
                else:
                    rc = lib.axon_start_nrt_profile(None, 0)
                if rc != 0:
                    raise RuntimeError(f"axon_start_nrt_profile rc={rc}")
                try:
                    yield
                finally:
                    n = lib.axon_stop_nrt_profile(str(output_dir).encode())
                    print(f"ntff profile: {n} file(s) -> {output_dir}")

    mod = types.ModuleType("antenv.axon_hooks")
    mod.get_axon_ntff_profile_hook = lambda: hook
    mod.set_axon_ntff_profile_hook = lambda h: None
    sys.modules["antenv.axon_hooks"] = mod


_install_ntff_hook()

F32 = mybir.dt.float32
F16 = mybir.dt.float16
BF16 = mybir.dt.bfloat16
U8 = mybir.dt.uint8
AF = mybir.ActivationFunctionType
ALU = mybir.AluOpType

DIM = 1024
HEADS = 16
DH = 64
B = 2
N = 1024          # query tokens
CTX = 1024        # context tokens
J = CTX + N       # kv length
HPC = 4           # heads per core
INNER_C = HPC * DH  # 256 per-core inner width
SCALE = DH ** -0.5
LN_EPS = 1e-5

N_CORES = 8
NT = N // 128      # 8 query-token tiles
JT = J // 128      # 16 kv tiles
DT = DIM // 128    # 8 d-chunks

# per-jc valid widths (queries lo(jc)..1024) and packed column offsets
LO = [0 if jc <= 8 else 128 * (jc - 8) for jc in range(JT)]
W = [N - LO[jc] for jc in range(JT)]
OFF = [0] * JT
for _jc in range(1, JT):
    OFF[_jc] = OFF[_jc - 1] + W[_jc - 1]
TOTW = OFF[-1] + W[-1]          # 12800 packed columns per head
# exp chunks (jc ranges): 4096/4096/3328/1280 packed cols
CHUNKS = [(0, 4), (4, 8), (8, 12), (12, 16)]
SLOTW = 4096


def _build_program():
    nc = bacc.Bacc(
        "TRN2",
        target_bir_lowering=False,
        debug=False,
        enable_asserts=False,
        num_devices=N_CORES,
    )
    # normalized activations, d-major: chunk dc is [128, J] = x̂T rows 128dc..
    xt = nc.dram_tensor("xt", [128, DT * J], BF16, kind="ExternalInput").ap()
    # weights packed partition-major on host: [128, DT*INNER_C]
    wq = nc.dram_tensor("wq", [128, DT * INNER_C], BF16, kind="ExternalInput").ap()
    wk = nc.dram_tensor("wk", [128, DT * INNER_C], BF16, kind="ExternalInput").ap()
    wv = nc.dram_tensor("wv", [128, DT * INNER_C], BF16, kind="ExternalInput").ap()
    wo = nc.dram_tensor("wo", [128, 2 * DIM], BF16, kind="ExternalInput").ap()
    # rope tables, d-major [128 = 2x(2x32) dh, J]; ssin has sign folded
    cosd = nc.dram_tensor("cosd", [128, J], BF16, kind="ExternalInput").ap()
    ssind = nc.dram_tensor("ssind", [128, J], BF16, kind="ExternalInput").ap()
    cmask = nc.dram_tensor("cmask", [128, CTX // 128], U8, kind="ExternalInput").ap()
    y = nc.dram_tensor("y", [N, DIM], F32, kind="ExternalOutput").ap()

    with tile.TileContext(nc) as tc:
        _kernel_body(tc, xt, wq, wk, wv, wo, cosd, ssind, cmask, y)
    nc.finalize()
    return nc


def _kernel_body(tc, xt, wq, wk, wv, wo, cosd, ssind, cmask, y):
    nc = tc.nc
    ctx_lp = nc.allow_low_precision(reason="bf16 matmul operands; fp32 PSUM accumulation")
    ctx_lp.__enter__()
    mm = nc.tensor.matmul

    with (
        tc.tile_pool(name="consts", bufs=1) as cpool,
        tc.tile_pool(name="qkv", bufs=1) as qkv_pool,
        tc.tile_pool(name="ptbuf", bufs=1) as pt_pool,
        tc.tile_pool(name="woin", bufs=1) as woin_pool,
        tc.tile_pool(name="outsb", bufs=4) as out_pool,
        tc.tile_pool(name="ropetmp", bufs=2) as rp_pool,
        tc.tile_pool(name="dens", bufs=1) as dens_pool,
    ):
        # ---- tiny consts (no DMA deps) -----------------------------------
        onespc = cpool.tile([128, HPC], F32, tag="onespc", name="onespc")
        nc.vector.memset(onespc[:], 1.0)
        warmc = cpool.tile([128, 512], BF16, tag="warmc", name="warmc")
        nc.gpsimd.memset(warmc[:], 0.25)
        # denominator-broadcast selector: row 0 -> partitions 0:64, row 32 -> 64:128
        sel2f = cpool.tile([64, 128], F32, tag="sel2f", name="sel2f")
        nc.vector.memset(sel2f[:], 0.0)
        nc.vector.memset(sel2f[0:1, 0:64], 1.0)
        nc.vector.memset(sel2f[32:33, 64:128], 1.0)
        sel2 = cpool.tile([64, 128], BF16, tag="sel2", name="sel2")
        nc.vector.tensor_copy(sel2[:], sel2f[:])
        # dens rows (f32): rows 0/32 valid per use; init 1.0 so rcp of the
        # unused rows stays finite
        dens = dens_pool.tile([64, N], F32, tag="dens", name="dens")
        nc.vector.memset(dens[:], 1.0)
        rcpf = dens_pool.tile([64, N], F32, tag="rcpf", name="rcpf")
        nc.vector.memset(rcpf[:], 1.0)
        rcpb = dens_pool.tile([64, N], BF16, tag="rcpb", name="rcpb")

        # ---- PE warmup: ~10 junk matmuls flip HAM to 8/8 by ~3.5us -------
        with tc.tile_pool(name="warm_psum", bufs=1, space="PSUM") as wpsum:
            wps = wpsum.tile([128, 512], F32, tag="wps", name="wps")
            for _ in range(40):
                mm(wps[:], warmc[:, 0:128], warmc[:], start=True, stop=True)

        # ---- DMA issue: all on sync, in priority order -------------------
        # (concurrent dma_starts fair-share HBM bandwidth across queues, so
        #  the critical-path transfers must be issued first)
        # projection-phase inputs live in a manually-scoped pool that is
        # released after pair-0 attention so pair-1's P buffer can reuse it
        proj_cm = tc.tile_pool(name="projbuf", bufs=1)
        proj_pool = proj_cm.__enter__()
        mu = cpool.tile([128, CTX // 128], U8, tag="mu8", name="mu8")
        nc.sync.dma_start(mu[:], cmask[:])
        wvt = proj_pool.tile([128, DT * INNER_C], BF16, tag="wvt", name="wvt")
        nc.sync.dma_start(wvt[:], wv[:])
        # xt halves: one 3D-AP DMA each ([128, dc, 1024])
        xth = [proj_pool.tile([128, DT * 1024], BF16, tag=f"xth{hf}",
                              name=f"xth{hf}") for hf in range(2)]
        nc.sync.dma_start(xth[0][:], xt[:, 0:DT * 1024])
        wkt = proj_pool.tile([128, DT * INNER_C], BF16, tag="wkt", name="wkt")
        nc.sync.dma_start(wkt[:], wk[:])
        cosT = proj_pool.tile([128, J], BF16, tag="cosT", name="cosT")
        nc.sync.dma_start(cosT[:], cosd[:])
        ssinT = proj_pool.tile([128, J], BF16, tag="ssinT", name="ssinT")
        nc.sync.dma_start(ssinT[:], ssind[:])
        nc.sync.dma_start(xth[1][:], xt[:, DT * 1024:DT * 2048])
        wqt = proj_pool.tile([128, DT * INNER_C], BF16, tag="wqt", name="wqt")
        nc.sync.dma_start(wqt[:], wq[:])
        wot = cpool.tile([128, 2 * DIM], BF16, tag="wot", name="wot")
        nc.sync.dma_start(wot[:], wo[:])
        wo_t = [wot[:, DIM * i:DIM * (i + 1)] for i in range(2)]

        maskf = cpool.tile([128, CTX // 128], F32, tag="maskf", name="maskf")
        nc.vector.tensor_copy(maskf[:], mu[:])

        def xt_view(dc, col0, width):
            hf, off = divmod(col0, 1024)
            assert off + width <= 1024
            return xth[hf][:, 1024 * dc + off:1024 * dc + off + width]

        # ---- long-lived activation tiles --------------------------------
        qT = [qkv_pool.tile([128, N], BF16, tag=f"qT{i}", name=f"qT{i}") for i in range(2)]
        kT = [qkv_pool.tile([128, J], BF16, tag=f"kT{i}", name=f"kT{i}") for i in range(2)]
        vaug = [qkv_pool.tile([128, HPC * (DH + 1)], BF16, tag=f"va{j}", name=f"va{j}")
                for j in range(JT)]
        woin = [woin_pool.tile([128, N], BF16, tag=f"woin{i}", name=f"woin{i}")
                for i in range(2)]
        # packed P (exp'd, bf16): head hh of the current pair at cols
        # [TOTW*hh, TOTW*(hh+1)); single tile so exp can write both heads
        # through one strided AP
        ptall_a = pt_pool.tile([128, 2 * TOTW], BF16, tag="ptall", name="ptall")
        pt_a = (ptall_a, ptall_a[:].rearrange("p (h c) -> p h c", h=2))

        # ---- phase P1: V projection (token-major) -----------------------
        # va[kv, h, 0:64] = V * mask(kv);  va[kv, h, 64] = mask(kv)
        with (
            tc.tile_pool(name="v_psum", bufs=3, space="PSUM") as v_psum,
        ):
            for m in range(JT):
                ps = v_psum.tile([128, INNER_C], F32, tag="vp", name="vp")
                for dc in range(DT):
                    mm(ps[:], xt_view(dc, 128 * m, 128),
                       wvt[:, INNER_C * dc:INNER_C * (dc + 1)],
                       start=(dc == 0), stop=(dc == DT - 1))
                va = vaug[m][:].rearrange("p (h f) -> p h f", h=HPC)
                psv = ps[:].rearrange("p (h f) -> p h f", h=HPC)
                if m < CTX // 128:
                    nc.vector.tensor_scalar(
                        va[:, :, 0:DH], psv, scalar1=maskf[:, m:m + 1],
                        scalar2=None, op0=ALU.mult)
                    nc.vector.tensor_scalar(
                        va[:, :, DH:DH + 1],
                        onespc[:].rearrange("p (h o) -> p h o", o=1),
                        scalar1=maskf[:, m:m + 1], scalar2=None, op0=ALU.mult)
                else:
                    nc.vector.tensor_copy(va[:, :, 0:DH], psv)
                    nc.vector.tensor_copy(
                        va[:, :, DH:DH + 1],
                        onespc[:].rearrange("p (h o) -> p h o", o=1))

        # ---- K/Q projection + rope --------------------------------------
        def proj_rope(ps_alloc, w, ih, src0, pos0, dst, dst0):
            """d-major projection + rope into dst[:, dst0:dst0+N]."""
            psc = rp_pool.tile([128, N], BF16, tag="psc", name="psc")
            for h2 in range(2):
                ps = ps_alloc()
                for dc in range(DT):
                    mm(ps[:],
                       w[:, INNER_C * dc + 128 * ih:INNER_C * dc + 128 * (ih + 1)],
                       xt_view(dc, src0 + 512 * h2, 512),
                       start=(dc == 0), stop=(dc == DT - 1))
                # psum evac on DVE (keep ACT free for exp)
                nc.vector.tensor_copy(psc[:, 512 * h2:512 * (h2 + 1)], ps[:])
            c1 = rp_pool.tile([128, N], BF16, tag="c1", name="c1")
            nc.vector.tensor_mul(c1[:], psc[:], cosT[:, pos0:pos0 + N])
            ts = rp_pool.tile([128, N], BF16, tag="ts", name="ts")
            shuf_engs = (nc.sync, nc.scalar, nc.gpsimd, nc.sync)
            for blk in range(4):
                sb = blk ^ 1
                shuf_engs[blk].dma_start(ts[32 * blk:32 * (blk + 1), :],
                                         psc[32 * sb:32 * (sb + 1), :])
            c2 = rp_pool.tile([128, N], BF16, tag="c2", name="c2")
            nc.vector.tensor_mul(c2[:], ts[:], ssinT[:, pos0:pos0 + N])
            nc.vector.tensor_add(dst[:, dst0:dst0 + N], c1[:], c2[:])

        def kq_pair(ps_alloc, ih):
            proj_rope(ps_alloc, wqt, ih, CTX, CTX, qT[ih], 0)
            proj_rope(ps_alloc, wkt, ih, 0, 0, kT[ih], 0)
            proj_rope(ps_alloc, wkt, ih, N, N, kT[ih], N)

        # ---- attention helpers ------------------------------------------
        def attention_pair(ih, sim_psum, pv_psum, pt, interleave,
                           post_alpha=None, resim=False):
            """Attention for head pair ih. `interleave`: callables issued
            at jc 3/7/11; post_alpha: issued after the first-half
            normalization (jc 11)."""
            ktp, qtp = kT[ih], qT[ih]
            ptall, ptv = pt
            pvh = {}
            for hh in range(2):
                for nh in range(2):
                    pvh[(hh, nh)] = pv_psum.tile(
                        [65, 512], F32, tag=f"pv{hh}{nh}", name=f"pv{hh}{nh}")

            def pv_mm(hh, jc, nh):
                lo, off = LO[jc], OFF[jc]
                a = max(lo, 512 * nh)
                b = 512 * (nh + 1)
                if a >= b:
                    return
                h = 2 * ih + hh
                p0 = TOTW * hh + off
                mm(pvh[(hh, nh)][0:65, a - 512 * nh:b - 512 * nh],
                   vaug[jc][:, 65 * h:65 * h + 65],
                   ptall[:, p0 + (a - lo):p0 + (b - lo)],
                   start=(jc == 0),
                   stop=(jc == (11 if nh == 0 else 15)))

            pvsbs = {}

            def den_part(nh):
                # DVE-only: evacuate pv numerators + denominators, start the
                # reciprocal chain; no PE work so the exp stream never waits
                cslice = slice(512 * nh, 512 * nh + 512)
                pvsb = rp_pool.tile([128, 512], F32, tag="pvsb", name="pvsb")
                pvsbs[nh] = pvsb
                for hh in range(2):
                    nc.vector.tensor_copy(pvsb[64 * hh:64 * hh + 64, :],
                                          pvh[(hh, nh)][0:64, :])
                    nc.vector.tensor_copy(dens[32 * hh:32 * hh + 1, cslice],
                                          pvh[(hh, nh)][64:65, :])
                nc.vector.reciprocal_approx_fast(rcpf[0:33, cslice],
                                                 dens[0:33, cslice])
                nc.vector.tensor_copy(rcpb[0:33, cslice], rcpf[0:33, cslice])

            def bc_part(nh):
                cslice = slice(512 * nh, 512 * nh + 512)
                bct = sim_psum.tile([128, 1024], F32, tag="simg", name="simg")
                bc = bct[:, 0:512]
                mm(bc, sel2[0:33, :], rcpb[0:33, cslice],
                   start=True, stop=True)
                nc.vector.tensor_mul(woin[ih][:, cslice], pvsbs[nh][:], bc)

            n_inter = 0
            for jc in range(JT):
                lo, w_ = LO[jc], W[jc]
                for seg0 in range(0, w_, 512):
                    seg1 = min(seg0 + 512, w_)
                    sw = seg1 - seg0
                    grp = sim_psum.tile([128, 1024], F32, tag="simg",
                                        name="simg")
                    gv = grp[:].rearrange("p (h c) -> p h c", h=2)
                    reps = 2 if resim else 1
                    for _ in range(reps):
                        for hh in range(2):
                            mm(grp[:, 512 * hh:512 * hh + sw],
                               ktp[64 * hh:64 * hh + 64,
                                   128 * jc:128 * (jc + 1)],
                               qtp[64 * hh:64 * hh + 64, lo + seg0:lo + seg1],
                               start=True, stop=True)
                    po = OFF[jc] + seg0
                    nc.scalar.activation(ptv[:, :, po:po + sw],
                                         gv[:, :, 0:sw], AF.Exp)
                    if jc >= 8 and seg0 == 0:
                        for hh in range(2):
                            nc.gpsimd.affine_select(
                                ptall[:, TOTW * hh + po:TOTW * hh + po + 128],
                                ptall[:, TOTW * hh + po:TOTW * hh + po + 128],
                                pattern=[[1, 128]], base=0,
                                channel_multiplier=-1,
                                compare_op=ALU.is_ge, fill=0.0)
                for hh in range(2):
                    if jc <= 11:
                        pv_mm(hh, jc, 0)
                    pv_mm(hh, jc, 1)
                if jc == 11:
                    den_part(0)

            den_part(1)
            return bc_part

        # ---- out projection (per query-token tile m) --------------------
        def out_proj(sim_psum, m, evac):
            for nh in range(2):
                pst = sim_psum.tile([128, 1024], F32, tag="simg", name="simg")
                ps = pst[:, 0:512]
                for kc in range(2):
                    mm(ps,
                       woin[kc][:, 128 * m:128 * (m + 1)],
                       wo_t[kc][:, 512 * nh:512 * (nh + 1)],
                       start=(kc == 0), stop=(kc == 1))
                ot = out_pool.tile([128, 512], F32, tag="osb", name="osb")
                if nh == 0:
                    nc.vector.tensor_copy(ot[:], ps)
                else:
                    nc.scalar.copy(ot[:], ps)
                nc.gpsimd.dma_start(
                    y[128 * m:128 * (m + 1), 512 * nh:512 * (nh + 1)], ot[:])

        # ---- main schedule ----------------------------------------------
        with tc.tile_pool(name="qk_psum", bufs=2, space="PSUM") as qk_psum:
            kq_pair(lambda: qk_psum.tile([128, 512], F32, tag="qkp",
                                         name="qkp"), 0)
        with (
            tc.tile_pool(name="sim_psum", bufs=2, space="PSUM") as sim_psum,
            tc.tile_pool(name="pv_psum", bufs=1, space="PSUM") as pv_psum,
        ):
            sim_alloc = lambda: sim_psum.tile([128, 1024], F32, tag="simg",
                                              name="simg")[:, 0:512]
            bc0 = attention_pair(0, sim_psum, pv_psum, pt_a, [])
            kq_pair(sim_alloc, 1)
            # pair-0 bc matmuls issue after KQ1's matmuls: the reciprocal
            # chain latency hides under the projection work
            bc0(0)
            bc0(1)
            proj_cm.__exit__(None, None, None)
            with tc.tile_pool(name="ptbuf2", bufs=1) as pt2_pool:
                ptall_b = pt2_pool.tile([128, 2 * TOTW], BF16, tag="ptallb",
                                        name="ptallb")
                pt_b = (ptall_b, ptall_b[:].rearrange("p (h c) -> p h c", h=2))
                bc1 = attention_pair(1, sim_psum, pv_psum, pt_b, [])
                bc1(0)
                bc1(1)
                for m in range(NT):
                    out_proj(sim_psum, m, None)
    ctx_lp.__exit__(None, None, None)


_NC = None
_LAST_RESULTS = None


def _get_program():
    global _NC
    if _NC is None:
        _NC = _build_program()
    return _NC


def _pack_rows(a):
    # [DT*128, W] -> [128, DT*W] partition-major
    k, w = a.shape[0] // 128, a.shape[1]
    return np.ascontiguousarray(
        a.reshape(k, 128, w).transpose(1, 0, 2).reshape(128, k * w))


def _bf16(a):
    return np.ascontiguousarray(a.astype(ml_dtypes.bfloat16))


def _ln(a, w, b):
    mu = a.mean(-1, keepdims=True)
    var = a.var(-1, keepdims=True)
    return (a - mu) / np.sqrt(var + LN_EPS) * w + b


def kernel(x, context, context_mask, rotary_pos_emb, norm_w, norm_b,
           cnorm_w, cnorm_b, Wq, Wkv, Wo, bo, _trace=False):
    global _LAST_RESULTS
    x = np.asarray(x, dtype=np.float32)
    context = np.asarray(context, dtype=np.float32)
    rot = np.asarray(rotary_pos_emb, dtype=np.float32)

    xn = _ln(x, np.asarray(norm_w, np.float32), np.asarray(norm_b, np.float32))
    cn = _ln(context, np.asarray(cnorm_w, np.float32),
             np.asarray(cnorm_b, np.float32))
    # [b] -> [128, DT*J] d-major packed bf16
    xt_pk = []
    for b in range(B):
        allx = np.concatenate([cn[b], xn[b]], axis=0)       # [J, DIM]
        pk = _pack_rows(np.ascontiguousarray(allx.T))       # [128, DT*J]
        # reorder to half-major: [h0: dc0..7 | h1: dc0..7], contiguous DMAs
        pk = pk.reshape(128, DT, 2, 1024).transpose(0, 2, 1, 3).reshape(
            128, DT * J)
        xt_pk.append(_bf16(pk))

    # rope tables d-major with sign folded into ssin
    cosT = np.tile(np.cos(rot).T, (2, 1))                   # [128, J]
    ssinT = np.sin(rot).T.copy()
    ssinT[:32] *= -1.0
    ssinT = np.tile(ssinT, (2, 1))
    cosT = _bf16(cosT)
    ssinT = _bf16(ssinT)

    Wq = np.asarray(Wq, dtype=np.float32) * SCALE
    Wkv = np.asarray(Wkv, dtype=np.float32)
    Wo = np.asarray(Wo, dtype=np.float32)
    mask_u8 = np.asarray(context_mask).reshape(B, CTX // 128, 128).view(np.uint8)
    mask_u8 = [np.ascontiguousarray(mask_u8[b].T) for b in range(B)]

    in_maps = []
    for c in range(N_CORES):
        b, hg = divmod(c, HEADS // HPC)
        lo = DH * HPC * hg
        in_maps.append({
            "xt": xt_pk[b],
            "wq": _bf16(_pack_rows(Wq[:, lo:lo + INNER_C])),
            "wk": _bf16(_pack_rows(Wkv[:, lo:lo + INNER_C])),
            "wv": _bf16(_pack_rows(Wkv[:, HEADS * DH + lo:HEADS * DH + lo + INNER_C])),
            "wo": _bf16(_pack_rows(Wo[lo:lo + INNER_C, :])),
            "cosd": cosT, "ssind": ssinT,
            "cmask": mask_u8[b],
        })

    nc = _get_program()
    res = bass_utils.run_bass_kernel_spmd(
        nc, in_maps, core_ids=list(range(N_CORES)), trace=_trace,
    )
    _LAST_RESULTS = res
    out = np.zeros((B, N, DIM), dtype=np.float32)
    for c in range(N_CORES):
        out[c // (HEADS // HPC)] += res.results[c]["y"]
    out += np.asarray(bo, dtype=np.float32)
    return out


# revision 26
# speedup vs baseline: 1.3705x; 1.0111x over previous
"""CausalPrefixAttention TRN2 Bass kernel (v3).

Full-input contract: kernel(**inputs) takes the complete tensors and returns
the complete [2, 1024, 1024] output. Internally shards (batch, head-group)
across 8 NeuronCores: core c handles batch c//4 and heads 4*(c%4) .. +4.

v3 changes vs v2 (165.6us baseline):
- PE warmup matmuls at t=0 so HAM un-throttles before real work.
- DMA issue spread across engine queues (each dma_start costs ~610ns of
  queue issue time; v2 serialized ~22 of them on sync).
- Context mask folded multiplicatively into the V tiles (V rows and the
  denominator ones-row), removing the per-jc exp bias -> exp calls can be
  batched: 3 big ACTIVATEs per head instead of 16 (saves the 352-cycle
  per-instruction ACT overhead; ACT is the attention-phase bottleneck).
- sim tiles for the two heads of a pair issued adjacently: K=64 matmuls
  auto-derive tile_position (0,0)/(64,0) -> concurrent row-tiled execution.
- sim PSUM evacuated by DVE into an fp16 staging ring (full-precision f32
  logits -> fp16), exp reads SBUF (64K free-dim limit, not 4K).
- Denominator reciprocal via reciprocal_approx_fast (1 custom DVE op,
  ~5x faster than the 8-pass reciprocal).
- pair-1 K/Q projections interleaved into pair-0's ACT-bound attention
  window; out-projection of token tiles 0-3 starts as soon as both pairs'
  first-half denominators are ready.
"""

import sys

for _p in ("/opt/trn_rl_repo", "/root/.axon_site/_ro/trn_rl_repo"):
    if _p not in sys.path:
        sys.path.append(_p)

import numpy as np
import ml_dtypes

import concourse.bass as bass
import concourse.mybir as mybir
import concourse.tile as tile
from concourse import bacc, bass_utils


def _install_ntff_hook():
    """Provide antenv.axon_hooks (NTFF profiling shim) if the image lacks it."""
    try:
        from antenv import axon_hooks  # noqa: F401
        return
    except ImportError:
        pass
    import contextlib
    import ctypes
    import os
    import types

    so_path = "/opt/axon/libaxon_pjrt.so"
    hook = None
    if os.path.exists(so_path):
        lib = ctypes.CDLL(so_path)
        if hasattr(lib, "axon_start_nrt_profile"):
            lib.axon_start_nrt_profile.argtypes = [
                ctypes.POINTER(ctypes.c_int64), ctypes.c_size_t]
            lib.axon_start_nrt_profile.restype = ctypes.c_int64
            lib.axon_stop_nrt_profile.argtypes = [ctypes.c_char_p]
            lib.axon_stop_nrt_profile.restype = ctypes.c_int64

            @contextlib.contextmanager
            def hook(output_dir, device_ids):
                import jax
                jax.devices()
                if device_ids:
                    ids = (ctypes.c_int64 * len(device_ids))(*device_ids)
                    rc = lib.axon_start_nrt_profile(ids, len(device_ids))
                else:
                    rc = lib.axon_start_nrt_profile(None, 0)
                if rc != 0:
                    raise RuntimeError(f"axon_start_nrt_profile rc={rc}")
                try:
                    yield
                finally:
                    n = lib.axon_stop_nrt_profile(str(output_dir).encode())
                    print(f"ntff profile: {n} file(s) -> {output_dir}")

    mod = types.ModuleType("antenv.axon_hooks")
    mod.get_axon_ntff_profile_hook = lambda: hook
    mod.set_axon_ntff_profile_hook = lambda h: None
    sys.modules["antenv.axon_hooks"] = mod


_install_ntff_hook()

F32 = mybir.dt.float32
F16 = mybir.dt.float16
BF16 = mybir.dt.bfloat16
U8 = mybir.dt.uint8
AF = mybir.ActivationFunctionType
ALU = mybir.AluOpType

DIM = 1024
HEADS = 16
DH = 64
B = 2
N = 1024          # query tokens
CTX = 1024        # context tokens
J = CTX + N       # kv length
HPC = 4           # heads per core
INNER_C = HPC * DH  # 256 per-core inner width
SCALE = DH ** -0.5
LN_EPS = 1e-5

N_CORES = 8
NT = N // 128      # 8 query-token tiles
JT = J // 128      # 16 kv tiles
DT = DIM // 128    # 8 d-chunks

# per-jc valid widths (queries lo(jc)..1024) and packed column offsets
LO = [0 if jc <= 8 else 128 * (jc - 8) for jc in range(JT)]
W = [N - LO[jc] for jc in range(JT)]
OFF = [0] * JT
for _jc in range(1, JT):
    OFF[_jc] = OFF[_jc - 1] + W[_jc - 1]
TOTW = OFF[-1] + W[-1]          # 12800 packed columns per head
# exp chunks (jc ranges): 4096/4096/3328/1280 packed cols
CHUNKS = [(0, 4), (4, 8), (8, 12), (12, 16)]
SLOTW = 4096


def _build_program():
    nc = bacc.Bacc(
        "TRN2",
        target_bir_lowering=False,
        debug=False,
        enable_asserts=False,
        num_devices=N_CORES,
    )
    # normalized activations, d-major: chunk dc is [128, J] = x̂T rows 128dc..
    xt = nc.dram_tensor("xt", [128, DT * J], BF16, kind="ExternalInput").ap()
    # weights packed partition-major on host: [128, DT*INNER_C]
    wq = nc.dram_tensor("wq", [128, DT * INNER_C], BF16, kind="ExternalInput").ap()
    wk = nc.dram_tensor("wk", [128, DT * INNER_C], BF16, kind="ExternalInput").ap()
    wv = nc.dram_tensor("wv", [128, DT * INNER_C], BF16, kind="ExternalInput").ap()
    wo = nc.dram_tensor("wo", [128, 2 * DIM], BF16, kind="ExternalInput").ap()
    # rope tables, d-major [128 = 2x(2x32) dh, J]; ssin has sign folded
    cosd = nc.dram_tensor("cosd", [128, J], BF16, kind="ExternalInput").ap()
    ssind = nc.dram_tensor("ssind", [128, J], BF16, kind="ExternalInput").ap()
    cmask = nc.dram_tensor("cmask", [128, CTX // 128], U8, kind="ExternalInput").ap()
    y = nc.dram_tensor("y", [N, DIM], F32, kind="ExternalOutput").ap()

    with tile.TileContext(nc) as tc:
        _kernel_body(tc, xt, wq, wk, wv, wo, cosd, ssind, cmask, y)
    nc.finalize()
    return nc


def _kernel_body(tc, xt, wq, wk, wv, wo, cosd, ssind, cmask, y):
    nc = tc.nc
    ctx_lp = nc.allow_low_precision(reason="bf16 matmul operands; fp32 PSUM accumulation")
    ctx_lp.__enter__()
    mm = nc.tensor.matmul

    with (
        tc.tile_pool(name="consts", bufs=1) as cpool,
        tc.tile_pool(name="qkv", bufs=1) as qkv_pool,
        tc.tile_pool(name="ptbuf", bufs=1) as pt_pool,
        tc.tile_pool(name="woin", bufs=1) as woin_pool,
        tc.tile_pool(name="outsb", bufs=4) as out_pool,
        tc.tile_pool(name="ropetmp", bufs=2) as rp_pool,
        tc.tile_pool(name="dens", bufs=1) as dens_pool,
    ):
        # ---- tiny consts (no DMA deps) -----------------------------------
        onespc = cpool.tile([128, HPC], F32, tag="onespc", name="onespc")
        nc.vector.memset(onespc[:], 1.0)
        warmc = cpool.tile([128, 512], BF16, tag="warmc", name="warmc")
        nc.gpsimd.memset(warmc[:], 0.25)
        # denominator-broadcast selector: row 0 -> partitions 0:64, row 32 -> 64:128
        sel2f = cpool.tile([64, 128], F32, tag="sel2f", name="sel2f")
        nc.vector.memset(sel2f[:], 0.0)
        nc.vector.memset(sel2f[0:1, 0:64], 1.0)
        nc.vector.memset(sel2f[32:33, 64:128], 1.0)
        sel2 = cpool.tile([64, 128], BF16, tag="sel2", name="sel2")
        nc.vector.tensor_copy(sel2[:], sel2f[:])
        # dens rows (f32): rows 0/32 valid per use; init 1.0 so rcp of the
        # unused rows stays finite
        dens = dens_pool.tile([64, N], F32, tag="dens", name="dens")
        nc.vector.memset(dens[:], 1.0)
        rcpf = dens_pool.tile([64, N], F32, tag="rcpf", name="rcpf")
        nc.vector.memset(rcpf[:], 1.0)
        rcpb = dens_pool.tile([64, N], BF16, tag="rcpb", name="rcpb")

        # ---- PE warmup: ~10 junk matmuls flip HAM to 8/8 by ~3.5us -------
        with tc.tile_pool(name="warm_psum", bufs=1, space="PSUM") as wpsum:
            wps = wpsum.tile([128, 512], F32, tag="wps", name="wps")
            for _ in range(40):
                mm(wps[:], warmc[:, 0:128], warmc[:], start=True, stop=True)

        # ---- DMA issue: all on sync, in priority order -------------------
        # (concurrent dma_starts fair-share HBM bandwidth across queues, so
        #  the critical-path transfers must be issued first)
        # projection-phase inputs live in a manually-scoped pool that is
        # released after pair-0 attention so pair-1's P buffer can reuse it
        proj_cm = tc.tile_pool(name="projbuf", bufs=1)
        proj_pool = proj_cm.__enter__()
        mu = cpool.tile([128, CTX // 128], U8, tag="mu8", name="mu8")
        nc.sync.dma_start(mu[:], cmask[:])
        wvt = proj_pool.tile([128, DT * INNER_C], BF16, tag="wvt", name="wvt")
        nc.sync.dma_start(wvt[:], wv[:])
        # xt halves: one 3D-AP DMA each ([128, dc, 1024])
        xth = [proj_pool.tile([128, DT * 1024], BF16, tag=f"xth{hf}",
                              name=f"xth{hf}") for hf in range(2)]
        nc.sync.dma_start(xth[0][:], xt[:, 0:DT * 1024])
        wkt = proj_pool.tile([128, DT * INNER_C], BF16, tag="wkt", name="wkt")
        nc.sync.dma_start(wkt[:], wk[:])
        cosT = proj_pool.tile([128, J], BF16, tag="cosT", name="cosT")
        nc.sync.dma_start(cosT[:], cosd[:])
        ssinT = proj_pool.tile([128, J], BF16, tag="ssinT", name="ssinT")
        nc.sync.dma_start(ssinT[:], ssind[:])
        nc.sync.dma_start(xth[1][:], xt[:, DT * 1024:DT * 2048])
        wqt = proj_pool.tile([128, DT * INNER_C], BF16, tag="wqt", name="wqt")
        nc.sync.dma_start(wqt[:], wq[:])
        wot = cpool.tile([128, 2 * DIM], BF16, tag="wot", name="wot")
        nc.sync.dma_start(wot[:], wo[:])
        wo_t = [wot[:, DIM * i:DIM * (i + 1)] for i in range(2)]

        maskf = cpool.tile([128, CTX // 128], F32, tag="maskf", name="maskf")
        nc.vector.tensor_copy(maskf[:], mu[:])

        def xt_view(dc, col0, width):
            hf, off = divmod(col0, 1024)
            assert off + width <= 1024
            return xth[hf][:, 1024 * dc + off:1024 * dc + off + width]

        # ---- long-lived activation tiles --------------------------------
        qT = [qkv_pool.tile([128, N], BF16, tag=f"qT{i}", name=f"qT{i}") for i in range(2)]
        kT = [qkv_pool.tile([128, J], BF16, tag=f"kT{i}", name=f"kT{i}") for i in range(2)]
        vaug = [qkv_pool.tile([128, HPC * (DH + 1)], BF16, tag=f"va{j}", name=f"va{j}")
                for j in range(JT)]
        woin = [woin_pool.tile([128, N], BF16, tag=f"woin{i}", name=f"woin{i}")
                for i in range(2)]
        # packed P (exp'd, bf16): head hh of the current pair at cols
        # [TOTW*hh, TOTW*(hh+1)); single tile so exp can write both heads
        # through one strided AP
        ptall_a = pt_pool.tile([128, 2 * TOTW], BF16, tag="ptall", name="ptall")
        pt_a = (ptall_a, ptall_a[:].rearrange("p (h c) -> p h c", h=2))

        # ---- phase P1: V projection (token-major) -----------------------
        # va[kv, h, 0:64] = V * mask(kv);  va[kv, h, 64] = mask(kv)
        def v_proj_tile(v_psum, m):
            ps = v_psum.tile([128, INNER_C], F32, tag="vp", name="vp")
            for dc in range(DT):
                mm(ps[:], xt_view(dc, 128 * m, 128),
                   wvt[:, INNER_C * dc:INNER_C * (dc + 1)],
                   start=(dc == 0), stop=(dc == DT - 1))
            va = vaug[m][:].rearrange("p (h f) -> p h f", h=HPC)
            psv = ps[:].rearrange("p (h f) -> p h f", h=HPC)
            if m < CTX // 128:
                nc.vector.tensor_scalar(
                    va[:, :, 0:DH], psv, scalar1=maskf[:, m:m + 1],
                    scalar2=None, op0=ALU.mult)
                nc.vector.tensor_scalar(
                    va[:, :, DH:DH + 1],
                    onespc[:].rearrange("p (h o) -> p h o", o=1),
                    scalar1=maskf[:, m:m + 1], scalar2=None, op0=ALU.mult)
            else:
                nc.vector.tensor_copy(va[:, :, 0:DH], psv)
                nc.vector.tensor_copy(
                    va[:, :, DH:DH + 1],
                    onespc[:].rearrange("p (h o) -> p h o", o=1))

        # ---- K/Q projection + rope --------------------------------------
        def proj_rope(ps_alloc, w, ih, src0, pos0, dst, dst0):
            """d-major projection + rope into dst[:, dst0:dst0+N]."""
            psc = rp_pool.tile([128, N], BF16, tag="psc", name="psc")
            for h2 in range(2):
                ps = ps_alloc()
                for dc in range(DT):
                    mm(ps[:],
                       w[:, INNER_C * dc + 128 * ih:INNER_C * dc + 128 * (ih + 1)],
                       xt_view(dc, src0 + 512 * h2, 512),
                       start=(dc == 0), stop=(dc == DT - 1))
                # psum evac on DVE (keep ACT free for exp)
                nc.vector.tensor_copy(psc[:, 512 * h2:512 * (h2 + 1)], ps[:])
            c1 = rp_pool.tile([128, N], BF16, tag="c1", name="c1")
            nc.vector.tensor_mul(c1[:], psc[:], cosT[:, pos0:pos0 + N])
            ts = rp_pool.tile([128, N], BF16, tag="ts", name="ts")
            shuf_engs = (nc.sync, nc.scalar, nc.gpsimd, nc.sync)
            for blk in range(4):
                sb = blk ^ 1
                shuf_engs[blk].dma_start(ts[32 * blk:32 * (blk + 1), :],
                                         psc[32 * sb:32 * (sb + 1), :])
            c2 = rp_pool.tile([128, N], BF16, tag="c2", name="c2")
            nc.vector.tensor_mul(c2[:], ts[:], ssinT[:, pos0:pos0 + N])
            nc.vector.tensor_add(dst[:, dst0:dst0 + N], c1[:], c2[:])

        def kq_pair(ps_alloc, ih):
            proj_rope(ps_alloc, wqt, ih, CTX, CTX, qT[ih], 0)
            proj_rope(ps_alloc, wkt, ih, 0, 0, kT[ih], 0)
            proj_rope(ps_alloc, wkt, ih, N, N, kT[ih], N)

        # ---- attention helpers ------------------------------------------
        def attention_pair(ih, sim_psum, pv_psum, pt, interleave,
                           post_alpha=None, resim=False):
            """Attention for head pair ih. `interleave`: callables issued
            at jc 3/7/11; post_alpha: issued after the first-half
            normalization (jc 11)."""
            ktp, qtp = kT[ih], qT[ih]
            ptall, ptv = pt
            pvh = {}
            for hh in range(2):
                for nh in range(2):
                    pvh[(hh, nh)] = pv_psum.tile(
                        [65, 512], F32, tag=f"pv{hh}{nh}", name=f"pv{hh}{nh}")

            def pv_mm(hh, jc, nh):
                lo, off = LO[jc], OFF[jc]
                a = max(lo, 512 * nh)
                b = 512 * (nh + 1)
                if a >= b:
                    return
                h = 2 * ih + hh
                p0 = TOTW * hh + off
                mm(pvh[(hh, nh)][0:65, a - 512 * nh:b - 512 * nh],
                   vaug[jc][:, 65 * h:65 * h + 65],
                   ptall[:, p0 + (a - lo):p0 + (b - lo)],
                   start=(jc == 0),
                   stop=(jc == (11 if nh == 0 else 15)))

            pvsbs = {}

            def den_part(nh):
                # DVE-only: evacuate pv numerators + denominators, start the
                # reciprocal chain; no PE work so the exp stream never waits
                cslice = slice(512 * nh, 512 * nh + 512)
                pvsb = rp_pool.tile([128, 512], F32, tag="pvsb", name="pvsb")
                pvsbs[nh] = pvsb
                for hh in range(2):
                    nc.vector.tensor_copy(pvsb[64 * hh:64 * hh + 64, :],
                                          pvh[(hh, nh)][0:64, :])
                    nc.vector.tensor_copy(dens[32 * hh:32 * hh + 1, cslice],
                                          pvh[(hh, nh)][64:65, :])
                nc.vector.reciprocal_approx_fast(rcpf[0:33, cslice],
                                                 dens[0:33, cslice])
                nc.vector.tensor_copy(rcpb[0:33, cslice], rcpf[0:33, cslice])

            def bc_part(nh):
                cslice = slice(512 * nh, 512 * nh + 512)
                bct = sim_psum.tile([128, 1024], F32, tag="simg", name="simg")
                bc = bct[:, 0:512]
                mm(bc, sel2[0:33, :], rcpb[0:33, cslice],
                   start=True, stop=True)
                nc.vector.tensor_mul(woin[ih][:, cslice], pvsbs[nh][:], bc)

            n_inter = 0
            for jc in range(JT):
                lo, w_ = LO[jc], W[jc]
                for seg0 in range(0, w_, 512):
                    seg1 = min(seg0 + 512, w_)
                    sw = seg1 - seg0
                    grp = sim_psum.tile([128, 1024], F32, tag="simg",
                                        name="simg")
                    gv = grp[:].rearrange("p (h c) -> p h c", h=2)
                    reps = 2 if resim else 1
                    for _ in range(reps):
                        for hh in range(2):
                            mm(grp[:, 512 * hh:512 * hh + sw],
                               ktp[64 * hh:64 * hh + 64,
                                   128 * jc:128 * (jc + 1)],
                               qtp[64 * hh:64 * hh + 64, lo + seg0:lo + seg1],
                               start=True, stop=True)
                    po = OFF[jc] + seg0
                    nc.scalar.activation(ptv[:, :, po:po + sw],
                                         gv[:, :, 0:sw], AF.Exp)
                    if jc >= 8 and seg0 == 0:
                        for hh in range(2):
                            nc.gpsimd.affine_select(
                                ptall[:, TOTW * hh + po:TOTW * hh + po + 128],
                                ptall[:, TOTW * hh + po:TOTW * hh + po + 128],
                                pattern=[[1, 128]], base=0,
                                channel_multiplier=-1,
                                compare_op=ALU.is_ge, fill=0.0)
                for hh in range(2):
                    if jc <= 11:
                        pv_mm(hh, jc, 0)
                    pv_mm(hh, jc, 1)
                if jc == 11:
                    den_part(0)

            den_part(1)
            return bc_part

        # ---- out projection (per query-token tile m) --------------------
        def out_proj(sim_psum, m, evac):
            for nh in range(2):
                pst = sim_psum.tile([128, 1024], F32, tag="simg", name="simg")
                ps = pst[:, 0:512]
                for kc in range(2):
                    mm(ps,
                       woin[kc][:, 128 * m:128 * (m + 1)],
                       wo_t[kc][:, 512 * nh:512 * (nh + 1)],
                       start=(kc == 0), stop=(kc == 1))
                ot = out_pool.tile([128, 512], F32, tag="osb", name="osb")
                if nh == 0:
                    nc.vector.tensor_copy(ot[:], ps)
                else:
                    nc.scalar.copy(ot[:], ps)
                seng = nc.gpsimd if (m + nh) % 2 == 0 else nc.sync
                seng.dma_start(
                    y[128 * m:128 * (m + 1), 512 * nh:512 * (nh + 1)], ot[:])

        # ---- main schedule ----------------------------------------------
        with tc.tile_pool(name="v_psum", bufs=3, space="PSUM") as v_psum:
            for m in range(12):
                v_proj_tile(v_psum, m)
            with tc.tile_pool(name="qk_psum", bufs=2, space="PSUM") as qk_psum:
                kq_pair(lambda: qk_psum.tile([128, 512], F32, tag="qkp",
                                             name="qkp"), 0)
                for m in range(12, JT):
                    v_proj_tile(v_psum, m)
        with (
            tc.tile_pool(name="sim_psum", bufs=2, space="PSUM") as sim_psum,
            tc.tile_pool(name="pv_psum", bufs=1, space="PSUM") as pv_psum,
        ):
            sim_alloc = lambda: sim_psum.tile([128, 1024], F32, tag="simg",
                                              name="simg")[:, 0:512]
            bc0 = attention_pair(0, sim_psum, pv_psum, pt_a, [])
            kq_pair(sim_alloc, 1)
            # pair-0 bc matmuls issue after KQ1's matmuls: the reciprocal
            # chain latency hides under the projection work
            bc0(0)
            bc0(1)
            proj_cm.__exit__(None, None, None)
            with tc.tile_pool(name="ptbuf2", bufs=1) as pt2_pool:
                ptall_b = pt2_pool.tile([128, 2 * TOTW], BF16, tag="ptallb",
                                        name="ptallb")
                pt_b = (ptall_b, ptall_b[:].rearrange("p (h c) -> p h c", h=2))
                bc1 = attention_pair(1, sim_psum, pv_psum, pt_b, [])
                # out m0-3 only reads woin cols < 512 (first-half norm);
                # the second-half reciprocal chain hides under them
                bc1(0)
                for m in range(4):
                    out_proj(sim_psum, m, None)
                bc1(1)
                for m in range(4, NT):
                    out_proj(sim_psum, m, None)
    ctx_lp.__exit__(None, None, None)


_NC = None
_LAST_RESULTS = None


def _get_program():
    global _NC
    if _NC is None:
        _NC = _build_program()
    return _NC


def _pack_rows(a):
    # [DT*128, W] -> [128, DT*W] partition-major
    k, w = a.shape[0] // 128, a.shape[1]
    return np.ascontiguousarray(
        a.reshape(k, 128, w).transpose(1, 0, 2).reshape(128, k * w))


def _bf16(a):
    return np.ascontiguousarray(a.astype(ml_dtypes.bfloat16))


def _ln(a, w, b):
    mu = a.mean(-1, keepdims=True)
    var = a.var(-1, keepdims=True)
    return (a - mu) / np.sqrt(var + LN_EPS) * w + b


def kernel(x, context, context_mask, rotary_pos_emb, norm_w, norm_b,
           cnorm_w, cnorm_b, Wq, Wkv, Wo, bo, _trace=False):
    global _LAST_RESULTS
    x = np.asarray(x, dtype=np.float32)
    context = np.asarray(context, dtype=np.float32)
    rot = np.asarray(rotary_pos_emb, dtype=np.float32)

    xn = _ln(x, np.asarray(norm_w, np.float32), np.asarray(norm_b, np.float32))
    cn = _ln(context, np.asarray(cnorm_w, np.float32),
             np.asarray(cnorm_b, np.float32))
    # [b] -> [128, DT*J] d-major packed bf16
    xt_pk = []
    for b in range(B):
        allx = np.concatenate([cn[b], xn[b]], axis=0)       # [J, DIM]
        pk = _pack_rows(np.ascontiguousarray(allx.T))       # [128, DT*J]
        # reorder to half-major: [h0: dc0..7 | h1: dc0..7], contiguous DMAs
        pk = pk.reshape(128, DT, 2, 1024).transpose(0, 2, 1, 3).reshape(
            128, DT * J)
        xt_pk.append(_bf16(pk))

    # rope tables d-major with sign folded into ssin
    cosT = np.tile(np.cos(rot).T, (2, 1))                   # [128, J]
    ssinT = np.sin(rot).T.copy()
    ssinT[:32] *= -1.0
    ssinT = np.tile(ssinT, (2, 1))
    cosT = _bf16(cosT)
    ssinT = _bf16(ssinT)

    Wq = np.asarray(Wq, dtype=np.float32) * SCALE
    Wkv = np.asarray(Wkv, dtype=np.float32)
    Wo = np.asarray(Wo, dtype=np.float32)
    mask_u8 = np.asarray(context_mask).reshape(B, CTX // 128, 128).view(np.uint8)
    mask_u8 = [np.ascontiguousarray(mask_u8[b].T) for b in range(B)]

    in_maps = []
    for c in range(N_CORES):
        b, hg = divmod(c, HEADS // HPC)
        lo = DH * HPC * hg
        in_maps.append({
            "xt": xt_pk[b],
            "wq": _bf16(_pack_rows(Wq[:, lo:lo + INNER_C])),
            "wk": _bf16(_pack_rows(Wkv[:, lo:lo + INNER_C])),
            "wv": _bf16(_pack_rows(Wkv[:, HEADS * DH + lo:HEADS * DH + lo + INNER_C])),
            "wo": _bf16(_pack_rows(Wo[lo:lo + INNER_C, :])),
            "cosd": cosT, "ssind": ssinT,
            "cmask": mask_u8[b],
        })

    nc = _get_program()
    res = bass_utils.run_bass_kernel_spmd(
        nc, in_maps, core_ids=list(range(N_CORES)), trace=_trace,
    )
    _LAST_RESULTS = res
    out = np.zeros((B, N, DIM), dtype=np.float32)
    for c in range(N_CORES):
        out[c // (HEADS // HPC)] += res.results[c]["y"]
    out += np.asarray(bo, dtype=np.float32)
    return out


# revision 27
# speedup vs baseline: 1.4598x; 1.0651x over previous
"""CausalPrefixAttention TRN2 Bass kernel (v3).

Full-input contract: kernel(**inputs) takes the complete tensors and returns
the complete [2, 1024, 1024] output. Internally shards (batch, head-group)
across 8 NeuronCores: core c handles batch c//4 and heads 4*(c%4) .. +4.

v3 changes vs v2 (165.6us baseline):
- PE warmup matmuls at t=0 so HAM un-throttles before real work.
- DMA issue spread across engine queues (each dma_start costs ~610ns of
  queue issue time; v2 serialized ~22 of them on sync).
- Context mask folded multiplicatively into the V tiles (V rows and the
  denominator ones-row), removing the per-jc exp bias -> exp calls can be
  batched: 3 big ACTIVATEs per head instead of 16 (saves the 352-cycle
  per-instruction ACT overhead; ACT is the attention-phase bottleneck).
- sim tiles for the two heads of a pair issued adjacently: K=64 matmuls
  auto-derive tile_position (0,0)/(64,0) -> concurrent row-tiled execution.
- sim PSUM evacuated by DVE into an fp16 staging ring (full-precision f32
  logits -> fp16), exp reads SBUF (64K free-dim limit, not 4K).
- Denominator reciprocal via reciprocal_approx_fast (1 custom DVE op,
  ~5x faster than the 8-pass reciprocal).
- pair-1 K/Q projections interleaved into pair-0's ACT-bound attention
  window; out-projection of token tiles 0-3 starts as soon as both pairs'
  first-half denominators are ready.
"""

import sys

for _p in ("/opt/trn_rl_repo", "/root/.axon_site/_ro/trn_rl_repo"):
    if _p not in sys.path:
        sys.path.append(_p)

import numpy as np
import ml_dtypes

import concourse.bass as bass
import concourse.mybir as mybir
import concourse.tile as tile
from concourse import bacc, bass_utils


def _install_ntff_hook():
    """Provide antenv.axon_hooks (NTFF profiling shim) if the image lacks it."""
    try:
        from antenv import axon_hooks  # noqa: F401
        return
    except ImportError:
        pass
    import contextlib
    import ctypes
    import os
    import types

    so_path = "/opt/axon/libaxon_pjrt.so"
    hook = None
    if os.path.exists(so_path):
        lib = ctypes.CDLL(so_path)
        if hasattr(lib, "axon_start_nrt_profile"):
            lib.axon_start_nrt_profile.argtypes = [
                ctypes.POINTER(ctypes.c_int64), ctypes.c_size_t]
            lib.axon_start_nrt_profile.restype = ctypes.c_int64
            lib.axon_stop_nrt_profile.argtypes = [ctypes.c_char_p]
            lib.axon_stop_nrt_profile.restype = ctypes.c_int64

            @contextlib.contextmanager
            def hook(output_dir, device_ids):
                import jax
                jax.devices()
                if device_ids:
                    ids = (ctypes.c_int64 * len(device_ids))(*device_ids)
                    rc = lib.axon_start_nrt_profile(ids, len(device_ids))
                else:
                    rc = lib.axon_start_nrt_profile(None, 0)
                if rc != 0:
                    raise RuntimeError(f"axon_start_nrt_profile rc={rc}")
                try:
                    yield
                finally:
                    n = lib.axon_stop_nrt_profile(str(output_dir).encode())
                    print(f"ntff profile: {n} file(s) -> {output_dir}")

    mod = types.ModuleType("antenv.axon_hooks")
    mod.get_axon_ntff_profile_hook = lambda: hook
    mod.set_axon_ntff_profile_hook = lambda h: None
    sys.modules["antenv.axon_hooks"] = mod


_install_ntff_hook()

F32 = mybir.dt.float32
F16 = mybir.dt.float16
BF16 = mybir.dt.bfloat16
U8 = mybir.dt.uint8
AF = mybir.ActivationFunctionType
ALU = mybir.AluOpType

DIM = 1024
HEADS = 16
DH = 64
B = 2
N = 1024          # query tokens
CTX = 1024        # context tokens
J = CTX + N       # kv length
HPC = 4           # heads per core
INNER_C = HPC * DH  # 256 per-core inner width
SCALE = DH ** -0.5
LN_EPS = 1e-5

N_CORES = 8
NT = N // 128      # 8 query-token tiles
JT = J // 128      # 16 kv tiles
DT = DIM // 128    # 8 d-chunks

# per-jc valid widths (queries lo(jc)..1024) and packed column offsets
LO = [0 if jc <= 8 else 128 * (jc - 8) for jc in range(JT)]
W = [N - LO[jc] for jc in range(JT)]
OFF = [0] * JT
for _jc in range(1, JT):
    OFF[_jc] = OFF[_jc - 1] + W[_jc - 1]
TOTW = OFF[-1] + W[-1]          # 12800 packed columns per head
# exp chunks (jc ranges): 4096/4096/3328/1280 packed cols
CHUNKS = [(0, 4), (4, 8), (8, 12), (12, 16)]
SLOTW = 4096


def _build_program():
    nc = bacc.Bacc(
        "TRN2",
        target_bir_lowering=False,
        debug=False,
        enable_asserts=False,
        num_devices=N_CORES,
    )
    # normalized activations, d-major: chunk dc is [128, J] = x̂T rows 128dc..
    xt = nc.dram_tensor("xt", [128, DT * J], BF16, kind="ExternalInput").ap()
    # weights packed partition-major on host: [128, DT*INNER_C]
    wq = nc.dram_tensor("wq", [128, DT * INNER_C], BF16, kind="ExternalInput").ap()
    wk = nc.dram_tensor("wk", [128, DT * INNER_C], BF16, kind="ExternalInput").ap()
    wv = nc.dram_tensor("wv", [128, DT * INNER_C], BF16, kind="ExternalInput").ap()
    wo = nc.dram_tensor("wo", [128, 2 * DIM], BF16, kind="ExternalInput").ap()
    # rope tables, d-major [128 = 2x(2x32) dh, J]; ssin has sign folded
    cosd = nc.dram_tensor("cosd", [128, J], BF16, kind="ExternalInput").ap()
    ssind = nc.dram_tensor("ssind", [128, J], BF16, kind="ExternalInput").ap()
    cmask = nc.dram_tensor("cmask", [128, CTX // 128], U8, kind="ExternalInput").ap()
    y = nc.dram_tensor("y", [N, DIM], F32, kind="ExternalOutput").ap()

    with tile.TileContext(nc) as tc:
        _kernel_body(tc, xt, wq, wk, wv, wo, cosd, ssind, cmask, y)
    nc.finalize()
    return nc


def _kernel_body(tc, xt, wq, wk, wv, wo, cosd, ssind, cmask, y):
    nc = tc.nc
    ctx_lp = nc.allow_low_precision(reason="bf16 matmul operands; fp32 PSUM accumulation")
    ctx_lp.__enter__()
    mm = nc.tensor.matmul

    with (
        tc.tile_pool(name="consts", bufs=1) as cpool,
        tc.tile_pool(name="qkv", bufs=1) as qkv_pool,
        tc.tile_pool(name="ptbuf", bufs=1) as pt_pool,
        tc.tile_pool(name="woin", bufs=1) as woin_pool,
        tc.tile_pool(name="outsb", bufs=4) as out_pool,
        tc.tile_pool(name="ropetmp", bufs=2) as rp_pool,
        tc.tile_pool(name="dens", bufs=1) as dens_pool,
    ):
        # ---- tiny consts (no DMA deps) -----------------------------------
        onespc = cpool.tile([128, HPC], F32, tag="onespc", name="onespc")
        nc.vector.memset(onespc[:], 1.0)
        warmc = cpool.tile([128, 512], BF16, tag="warmc", name="warmc")
        nc.gpsimd.memset(warmc[:], 0.25)
        # denominator-broadcast selector: row 0 -> partitions 0:64, row 32 -> 64:128
        sel2f = cpool.tile([64, 128], F32, tag="sel2f", name="sel2f")
        nc.vector.memset(sel2f[:], 0.0)
        nc.vector.memset(sel2f[0:1, 0:64], 1.0)
        nc.vector.memset(sel2f[32:33, 64:128], 1.0)
        sel2 = cpool.tile([64, 128], BF16, tag="sel2", name="sel2")
        nc.vector.tensor_copy(sel2[:], sel2f[:])
        # dens rows (f32): rows 0/32 valid per use; init 1.0 so rcp of the
        # unused rows stays finite
        dens = dens_pool.tile([64, N], F32, tag="dens", name="dens")
        nc.vector.memset(dens[:], 1.0)
        rcpf = dens_pool.tile([64, N], F32, tag="rcpf", name="rcpf")
        nc.vector.memset(rcpf[:], 1.0)
        rcpb = dens_pool.tile([64, N], BF16, tag="rcpb", name="rcpb")

        # ---- PE warmup: ~10 junk matmuls flip HAM to 8/8 by ~3.5us -------
        with tc.tile_pool(name="warm_psum", bufs=1, space="PSUM") as wpsum:
            wps = wpsum.tile([128, 512], F32, tag="wps", name="wps")
            for _ in range(40):
                mm(wps[:], warmc[:, 0:128], warmc[:], start=True, stop=True)

        # ---- DMA issue: all on sync, in priority order -------------------
        # (concurrent dma_starts fair-share HBM bandwidth across queues, so
        #  the critical-path transfers must be issued first)
        # projection-phase inputs live in a manually-scoped pool that is
        # released after pair-0 attention so pair-1's P buffer can reuse it
        proj_cm = tc.tile_pool(name="projbuf", bufs=1)
        proj_pool = proj_cm.__enter__()
        mu = cpool.tile([128, CTX // 128], U8, tag="mu8", name="mu8")
        nc.sync.dma_start(mu[:], cmask[:])
        wvt = proj_pool.tile([128, DT * INNER_C], BF16, tag="wvt", name="wvt")
        nc.sync.dma_start(wvt[:], wv[:])
        # xt halves: one 3D-AP DMA each ([128, dc, 1024])
        xth = [proj_pool.tile([128, DT * 1024], BF16, tag=f"xth{hf}",
                              name=f"xth{hf}") for hf in range(2)]
        nc.sync.dma_start(xth[0][:], xt[:, 0:DT * 1024])
        wkt = proj_pool.tile([128, DT * INNER_C], BF16, tag="wkt", name="wkt")
        nc.sync.dma_start(wkt[:], wk[:])
        cosT = proj_pool.tile([128, J], BF16, tag="cosT", name="cosT")
        nc.sync.dma_start(cosT[:], cosd[:])
        ssinT = proj_pool.tile([128, J], BF16, tag="ssinT", name="ssinT")
        nc.sync.dma_start(ssinT[:], ssind[:])
        nc.sync.dma_start(xth[1][:], xt[:, DT * 1024:DT * 2048])
        wqt = proj_pool.tile([128, DT * INNER_C], BF16, tag="wqt", name="wqt")
        nc.sync.dma_start(wqt[:], wq[:])
        wot = cpool.tile([128, 2 * DIM], BF16, tag="wot", name="wot")
        nc.sync.dma_start(wot[:], wo[:])
        wo_t = [wot[:, DIM * i:DIM * (i + 1)] for i in range(2)]

        maskf = cpool.tile([128, CTX // 128], F32, tag="maskf", name="maskf")
        nc.vector.tensor_copy(maskf[:], mu[:])

        def xt_view(dc, col0, width):
            hf, off = divmod(col0, 1024)
            assert off + width <= 1024
            return xth[hf][:, 1024 * dc + off:1024 * dc + off + width]

        # ---- long-lived activation tiles --------------------------------
        qT = [qkv_pool.tile([128, N], BF16, tag=f"qT{i}", name=f"qT{i}") for i in range(2)]
        kT = [qkv_pool.tile([128, J], BF16, tag=f"kT{i}", name=f"kT{i}") for i in range(2)]
        vaug = [qkv_pool.tile([128, HPC * (DH + 1)], BF16, tag=f"va{j}", name=f"va{j}")
                for j in range(JT)]
        woin = [woin_pool.tile([128, N], BF16, tag=f"woin{i}", name=f"woin{i}")
                for i in range(2)]
        # packed P (exp'd, bf16): head hh of the current pair at cols
        # [TOTW*hh, TOTW*(hh+1)); single tile so exp can write both heads
        # through one strided AP
        ptall_a = pt_pool.tile([128, 2 * TOTW], BF16, tag="ptall", name="ptall")
        pt_a = (ptall_a, ptall_a[:].rearrange("p (h c) -> p h c", h=2))

        # ---- phase P1: V projection (token-major) -----------------------
        # va[kv, h, 0:64] = V * mask(kv);  va[kv, h, 64] = mask(kv)
        def v_proj_tile(v_psum, m):
            ps = v_psum.tile([128, INNER_C], F32, tag="vp", name="vp")
            for dc in range(DT):
                mm(ps[:], xt_view(dc, 128 * m, 128),
                   wvt[:, INNER_C * dc:INNER_C * (dc + 1)],
                   start=(dc == 0), stop=(dc == DT - 1))
            va = vaug[m][:].rearrange("p (h f) -> p h f", h=HPC)
            psv = ps[:].rearrange("p (h f) -> p h f", h=HPC)
            if m < CTX // 128:
                nc.scalar.activation(va[:, :, 0:DH], psv, AF.Copy,
                                     scale=maskf[:, m:m + 1])
                nc.vector.tensor_scalar(
                    va[:, :, DH:DH + 1],
                    onespc[:].rearrange("p (h o) -> p h o", o=1),
                    scalar1=maskf[:, m:m + 1], scalar2=None, op0=ALU.mult)
            else:
                nc.scalar.copy(va[:, :, 0:DH], psv)
                nc.vector.tensor_copy(
                    va[:, :, DH:DH + 1],
                    onespc[:].rearrange("p (h o) -> p h o", o=1))

        # ---- K/Q projection + rope --------------------------------------
        def proj_rope(ps_alloc, w, ih, src0, pos0, dst, dst0):
            """d-major projection + rope into dst[:, dst0:dst0+N]."""
            psc = rp_pool.tile([128, N], BF16, tag="psc", name="psc")
            for h2 in range(2):
                ps = ps_alloc()
                for dc in range(DT):
                    mm(ps[:],
                       w[:, INNER_C * dc + 128 * ih:INNER_C * dc + 128 * (ih + 1)],
                       xt_view(dc, src0 + 512 * h2, 512),
                       start=(dc == 0), stop=(dc == DT - 1))
                # psum evac on ACT (idle during projection windows)
                nc.scalar.copy(psc[:, 512 * h2:512 * (h2 + 1)], ps[:])
            c1 = rp_pool.tile([128, N], BF16, tag="c1", name="c1")
            nc.vector.tensor_mul(c1[:], psc[:], cosT[:, pos0:pos0 + N])
            ts = rp_pool.tile([128, N], BF16, tag="ts", name="ts")
            shuf_engs = (nc.sync, nc.scalar, nc.gpsimd, nc.sync)
            for blk in range(4):
                sb = blk ^ 1
                shuf_engs[blk].dma_start(ts[32 * blk:32 * (blk + 1), :],
                                         psc[32 * sb:32 * (sb + 1), :])
            c2 = rp_pool.tile([128, N], BF16, tag="c2", name="c2")
            nc.vector.tensor_mul(c2[:], ts[:], ssinT[:, pos0:pos0 + N])
            nc.vector.tensor_add(dst[:, dst0:dst0 + N], c1[:], c2[:])

        def kq_pair(ps_alloc, ih):
            proj_rope(ps_alloc, wqt, ih, CTX, CTX, qT[ih], 0)
            proj_rope(ps_alloc, wkt, ih, 0, 0, kT[ih], 0)
            proj_rope(ps_alloc, wkt, ih, N, N, kT[ih], N)

        # ---- attention helpers ------------------------------------------
        def attention_pair(ih, sim_psum, pv_psum, pt, interleave,
                           post_alpha=None, resim=False):
            """Attention for head pair ih. `interleave`: callables issued
            at jc 3/7/11; post_alpha: issued after the first-half
            normalization (jc 11)."""
            ktp, qtp = kT[ih], qT[ih]
            ptall, ptv = pt
            pvh = {}
            for hh in range(2):
                for nh in range(2):
                    pvh[(hh, nh)] = pv_psum.tile(
                        [65, 512], F32, tag=f"pv{hh}{nh}", name=f"pv{hh}{nh}")

            def pv_mm(hh, jc, nh):
                lo, off = LO[jc], OFF[jc]
                a = max(lo, 512 * nh)
                b = 512 * (nh + 1)
                if a >= b:
                    return
                h = 2 * ih + hh
                p0 = TOTW * hh + off
                mm(pvh[(hh, nh)][0:65, a - 512 * nh:b - 512 * nh],
                   vaug[jc][:, 65 * h:65 * h + 65],
                   ptall[:, p0 + (a - lo):p0 + (b - lo)],
                   start=(jc == 0),
                   stop=(jc == (11 if nh == 0 else 15)))

            pvsbs = {}

            def den_part(nh, act_ok=False):
                # evacuate pv numerators + denominators, start the
                # reciprocal chain; no PE work so the exp stream never waits
                cslice = slice(512 * nh, 512 * nh + 512)
                pvsb = rp_pool.tile([128, 512], F32, tag="pvsb", name="pvsb")
                pvsbs[nh] = pvsb
                cp = nc.scalar.copy if act_ok else nc.vector.tensor_copy
                for hh in range(2):
                    cp(pvsb[64 * hh:64 * hh + 64, :],
                       pvh[(hh, nh)][0:64, :])
                    cp(dens[32 * hh:32 * hh + 1, cslice],
                       pvh[(hh, nh)][64:65, :])
                nc.vector.reciprocal_approx_fast(rcpf[0:33, cslice],
                                                 dens[0:33, cslice])
                nc.vector.tensor_copy(rcpb[0:33, cslice], rcpf[0:33, cslice])

            def bc_part(nh):
                cslice = slice(512 * nh, 512 * nh + 512)
                bct = sim_psum.tile([128, 1024], F32, tag="simg", name="simg")
                bc = bct[:, 0:512]
                mm(bc, sel2[0:33, :], rcpb[0:33, cslice],
                   start=True, stop=True)
                nc.vector.tensor_mul(woin[ih][:, cslice], pvsbs[nh][:], bc)

            n_inter = 0
            for jc in range(JT):
                lo, w_ = LO[jc], W[jc]
                for seg0 in range(0, w_, 512):
                    seg1 = min(seg0 + 512, w_)
                    sw = seg1 - seg0
                    grp = sim_psum.tile([128, 1024], F32, tag="simg",
                                        name="simg")
                    gv = grp[:].rearrange("p (h c) -> p h c", h=2)
                    reps = 2 if resim else 1
                    for _ in range(reps):
                        for hh in range(2):
                            mm(grp[:, 512 * hh:512 * hh + sw],
                               ktp[64 * hh:64 * hh + 64,
                                   128 * jc:128 * (jc + 1)],
                               qtp[64 * hh:64 * hh + 64, lo + seg0:lo + seg1],
                               start=True, stop=True)
                    po = OFF[jc] + seg0
                    nc.scalar.activation(ptv[:, :, po:po + sw],
                                         gv[:, :, 0:sw], AF.Exp)
                    if jc >= 8 and seg0 == 0:
                        for hh in range(2):
                            nc.gpsimd.affine_select(
                                ptall[:, TOTW * hh + po:TOTW * hh + po + 128],
                                ptall[:, TOTW * hh + po:TOTW * hh + po + 128],
                                pattern=[[1, 128]], base=0,
                                channel_multiplier=-1,
                                compare_op=ALU.is_ge, fill=0.0)
                for hh in range(2):
                    if jc <= 11:
                        pv_mm(hh, jc, 0)
                    pv_mm(hh, jc, 1)
                if jc == 11:
                    den_part(0)

            den_part(1, act_ok=True)
            return bc_part

        # ---- out projection (per query-token tile m) --------------------
        def out_proj(sim_psum, m, evac):
            for nh in range(2):
                pst = sim_psum.tile([128, 1024], F32, tag="simg", name="simg")
                ps = pst[:, 0:512]
                for kc in range(2):
                    mm(ps,
                       woin[kc][:, 128 * m:128 * (m + 1)],
                       wo_t[kc][:, 512 * nh:512 * (nh + 1)],
                       start=(kc == 0), stop=(kc == 1))
                ot = out_pool.tile([128, 512], F32, tag="osb", name="osb")
                if nh == 0:
                    nc.vector.tensor_copy(ot[:], ps)
                else:
                    nc.scalar.copy(ot[:], ps)
                seng = nc.gpsimd if (m + nh) % 2 == 0 else nc.sync
                seng.dma_start(
                    y[128 * m:128 * (m + 1), 512 * nh:512 * (nh + 1)], ot[:])

        # ---- main schedule ----------------------------------------------
        with tc.tile_pool(name="v_psum", bufs=3, space="PSUM") as v_psum:
            for m in range(12):
                v_proj_tile(v_psum, m)
            with tc.tile_pool(name="qk_psum", bufs=2, space="PSUM") as qk_psum:
                kq_pair(lambda: qk_psum.tile([128, 512], F32, tag="qkp",
                                             name="qkp"), 0)
                for m in range(12, JT):
                    v_proj_tile(v_psum, m)
        with (
            tc.tile_pool(name="sim_psum", bufs=2, space="PSUM") as sim_psum,
            tc.tile_pool(name="pv_psum", bufs=1, space="PSUM") as pv_psum,
        ):
            sim_alloc = lambda: sim_psum.tile([128, 1024], F32, tag="simg",
                                              name="simg")[:, 0:512]
            bc0 = attention_pair(0, sim_psum, pv_psum, pt_a, [])
            kq_pair(sim_alloc, 1)
            # pair-0 bc matmuls issue after KQ1's matmuls: the reciprocal
            # chain latency hides under the projection work
            bc0(0)
            bc0(1)
            proj_cm.__exit__(None, None, None)
            with tc.tile_pool(name="ptbuf2", bufs=1) as pt2_pool:
                ptall_b = pt2_pool.tile([128, 2 * TOTW], BF16, tag="ptallb",
                                        name="ptallb")
                pt_b = (ptall_b, ptall_b[:].rearrange("p (h c) -> p h c", h=2))
                bc1 = attention_pair(1, sim_psum, pv_psum, pt_b, [])
                # out m0-3 only reads woin cols < 512 (first-half norm);
                # the second-half reciprocal chain hides under them
                bc1(0)
                for m in range(4):
                    out_proj(sim_psum, m, None)
                bc1(1)
                for m in range(4, NT):
                    out_proj(sim_psum, m, None)
    ctx_lp.__exit__(None, None, None)


_NC = None
_LAST_RESULTS = None


def _get_program():
    global _NC
    if _NC is None:
        _NC = _build_program()
    return _NC


def _pack_rows(a):
    # [DT*128, W] -> [128, DT*W] partition-major
    k, w = a.shape[0] // 128, a.shape[1]
    return np.ascontiguousarray(
        a.reshape(k, 128, w).transpose(1, 0, 2).reshape(128, k * w))


def _bf16(a):
    return np.ascontiguousarray(a.astype(ml_dtypes.bfloat16))


def _ln(a, w, b):
    mu = a.mean(-1, keepdims=True)
    var = a.var(-1, keepdims=True)
    return (a - mu) / np.sqrt(var + LN_EPS) * w + b


def kernel(x, context, context_mask, rotary_pos_emb, norm_w, norm_b,
           cnorm_w, cnorm_b, Wq, Wkv, Wo, bo, _trace=False):
    global _LAST_RESULTS
    x = np.asarray(x, dtype=np.float32)
    context = np.asarray(context, dtype=np.float32)
    rot = np.asarray(rotary_pos_emb, dtype=np.float32)

    xn = _ln(x, np.asarray(norm_w, np.float32), np.asarray(norm_b, np.float32))
    cn = _ln(context, np.asarray(cnorm_w, np.float32),
             np.asarray(cnorm_b, np.float32))
    # [b] -> [128, DT*J] d-major packed bf16
    xt_pk = []
    for b in range(B):
        allx = np.concatenate([cn[b], xn[b]], axis=0)       # [J, DIM]
        pk = _pack_rows(np.ascontiguousarray(allx.T))       # [128, DT*J]
        # reorder to half-major: [h0: dc0..7 | h1: dc0..7], contiguous DMAs
        pk = pk.reshape(128, DT, 2, 1024).transpose(0, 2, 1, 3).reshape(
            128, DT * J)
        xt_pk.append(_bf16(pk))

    # rope tables d-major with sign folded into ssin
    cosT = np.tile(np.cos(rot).T, (2, 1))                   # [128, J]
    ssinT = np.sin(rot).T.copy()
    ssinT[:32] *= -1.0
    ssinT = np.tile(ssinT, (2, 1))
    cosT = _bf16(cosT)
    ssinT = _bf16(ssinT)

    Wq = np.asarray(Wq, dtype=np.float32) * SCALE
    Wkv = np.asarray(Wkv, dtype=np.float32)
    Wo = np.asarray(Wo, dtype=np.float32)
    mask_u8 = np.asarray(context_mask).reshape(B, CTX // 128, 128).view(np.uint8)
    mask_u8 = [np.ascontiguousarray(mask_u8[b].T) for b in range(B)]

    in_maps = []
    for c in range(N_CORES):
        b, hg = divmod(c, HEADS // HPC)
        lo = DH * HPC * hg
        in_maps.append({
            "xt": xt_pk[b],
            "wq": _bf16(_pack_rows(Wq[:, lo:lo + INNER_C])),
            "wk": _bf16(_pack_rows(Wkv[:, lo:lo + INNER_C])),
            "wv": _bf16(_pack_rows(Wkv[:, HEADS * DH + lo:HEADS * DH + lo + INNER_C])),
            "wo": _bf16(_pack_rows(Wo[lo:lo + INNER_C, :])),
            "cosd": cosT, "ssind": ssinT,
            "cmask": mask_u8[b],
        })

    nc = _get_program()
    res = bass_utils.run_bass_kernel_spmd(
        nc, in_maps, core_ids=list(range(N_CORES)), trace=_trace,
    )
    _LAST_RESULTS = res
    out = np.zeros((B, N, DIM), dtype=np.float32)
    for c in range(N_CORES):
        out[c // (HEADS // HPC)] += res.results[c]["y"]
    out += np.asarray(bo, dtype=np.float32)
    return out
